# revision 1
# baseline (speedup 1.0000x reference)
"""nn_BitConv: ternary 3x3 conv (stride 1, pad 1) + BatchNorm(eval) + SiLU
on 8 Trainium2 NeuronCores, data-parallel over the batch dimension.

Strategy
--------
Host (numpy, negligible cost): ternarize the weight exactly like the
reference (scale = 1/median|w|, w_q = clamp(round(w*scale))/scale) and keep
only the integer part t in {-1,0,+1} (exact in fp16); fold the 1/scale
factor and the BatchNorm affine into a single per-output-channel
scale/shift (a, b). Pre-transpose the weight into the tensor-engine
stationary layout and zero-pad x to 58x58 / cast to fp16 (same PE rate as
bf16 but 10 mantissa bits: products with ternary weights are exact; only
the fp16 rounding of x contributes error, ~2e-4 relative on the output).

Device (per core, 4 images): the 3x3 conv is 9 shifted matmuls x 2
C1-chunks of K=128 accumulated in PSUM. For each image, C2-chunk (2x128)
and 8-row output block (7 per image), 18 matmuls of [K=128, M=128] x
[128, N=8*56=448] accumulate one PSUM tile; a single ScalarE activation
applies Silu(a*z + b) fused, then the tile is DMA'd out. 1008 back-to-back
matmuls keep the PE warm; ACT/DMA run concurrently. Measured ~180-220 us
per core (burst vs sustained-clock), at the 16-bit PE roofline for the
14.8 GFLOP/core conv.
"""
import numpy as np
import concourse.bass as bass
from concourse import mybir
from concourse.bass_utils import run_bass_kernel_spmd
from concourse.tile import TileContext
from concourse.vector_clock import ScopedClock

X16 = mybir.dt.float16
F32 = mybir.dt.float32
NP_X16 = np.float16

N_CORES = 8
B, C, H, W = 32, 256, 56, 56
B_LOC = B // N_CORES
HP, WP = H + 2, W + 2
RB = 8            # output rows per PSUM tile (N = 8*56 = 448 <= 512)
NRB = H // RB


class _SplitDrainTC(TileContext):
    """This walrus build allows a single sync wait on the SP CTRL (Drain)
    instruction; split the Tile tail drain's waits across extra drains."""

    def _drain_and_barrier(self, tick_clock, wait_clock):
        drain_inst = self.nc.sync.drain()
        wait_clock.add_sem_waits(
            drain_inst.ins, ScopedClock({None: tick_clock.global_clock})
        )
        si = drain_inst.ins.sync_info
        waits = list(si.on_wait or []) if si is not None else []
        if len(waits) > 1:
            si.on_wait = waits[:1]
            for k in range(1, len(waits)):
                d2 = self.nc.sync.drain()
                si2 = d2.ins.sync_info
                if si2 is None:
                    d2.ins.sync_info = mybir.SyncInfo(
                        on_wait=[waits[k]], on_update=[]
                    )
                else:
                    si2.on_wait = [waits[k]]
        self.nc.all_engine_barrier()
        assert self.sems is not None
        popped = self.nc._tile_sem_poison_stack.pop()
        assert popped is self._sem_poison
        self.nc.clear_and_free_semaphores(list(self.sems.allocated().values()))
        self.nc.all_engine_barrier()


def split_sync_waits(nc, limit=1):
    """Hoist excess per-instruction sem waits onto same-engine nops (this
    walrus build allows only `limit` sync waits per instruction)."""
    builders = {
        mybir.EngineType.PE: nc.tensor,
        mybir.EngineType.Activation: nc.scalar,
        mybir.EngineType.DVE: nc.vector,
        mybir.EngineType.Pool: nc.gpsimd,
        mybir.EngineType.SP: nc.sync,
    }
    n_split = 0
    for f in nc.m.functions:
        for bb in f.blocks:
            insts = bb.instructions
            idx = 0
            while idx < len(insts):
                inst = insts[idx]
                si = inst.sync_info
                waits = list(si.on_wait) if (si is not None and si.on_wait) else []
                if len(waits) <= limit:
                    idx += 1
                    continue
                eng = inst.engine
                if eng not in builders:
                    raise RuntimeError(
                        f"split_sync_waits: no builder for engine {eng} "
                        f"on {inst.name} ({type(inst).__name__})"
                    )
                si.on_wait = waits[-limit:]
                carriers = []
                for w in waits[:-limit]:
                    nop = builders[eng].nop(nofuse=True)
                    ci = nop.ins
                    tail_bb = nc.cur_bb.bb
                    assert tail_bb.instructions[-1] is ci
                    tail_bb.instructions.pop()
                    ci.sync_info = mybir.SyncInfo(on_wait=[w], on_update=[])
                    carriers.append(ci)
                for k, ci in enumerate(carriers):
                    insts.insert(idx + k, ci)
                n_split += 1
                idx += len(carriers) + 1
    return n_split


def build_nc(b_loc=B_LOC, repeats=1, do_split=True):
    nc = bass.Bass()
    xp_d = nc.dram_tensor("xp", [b_loc, 2, 128, HP, WP], X16, kind="ExternalInput")
    wp_d = nc.dram_tensor("wp", [2, 128, 9, 2, 128], X16, kind="ExternalInput")
    ab_d = nc.dram_tensor("ab", [2, 128, 2], F32, kind="ExternalInput")
    out_d = nc.dram_tensor("out", [b_loc, 2, 128, H, W], F32, kind="ExternalOutput")

    with _SplitDrainTC(nc) as tc:
        with (
            tc.tile_pool(name="consts", bufs=1) as consts,
            tc.tile_pool(name="xpool", bufs=1) as xpool,
            tc.tile_pool(name="psum", bufs=8, space="PSUM") as psum,
            tc.tile_pool(name="opool", bufs=4) as opool,
        ):
            w_sb = []
            for i in range(2):
                w = consts.tile([128, 9, 2, 128], X16, tag=f"w{i}")
                nc.sync.dma_start(w[:], wp_d[i])
                w_sb.append(w)
            a_sb, b_sb = [], []
            for j in range(2):
                a = consts.tile([128, 1], F32, tag=f"a{j}")
                nc.sync.dma_start(a[:], ab_d[j, :, 0:1])
                a_sb.append(a)
                bt = consts.tile([128, 1], F32, tag=f"b{j}")
                nc.sync.dma_start(bt[:], ab_d[j, :, 1:2])
                b_sb.append(bt)
            x_sb = [[None] * 2 for _ in range(b_loc)]
            for n in range(b_loc):
                for i in range(2):
                    xt = xpool.tile([128, HP, WP], X16, tag=f"x{n}_{i}")
                    nc.sync.dma_start(xt[:], xp_d[n, i])
                    x_sb[n][i] = xt

            for _rep in range(repeats):
                for n in range(b_loc):
                    for j in range(2):
                        for r in range(NRB):
                            ps = psum.tile([128, RB, W], F32, tag="ps")
                            idx = 0
                            for ky in range(3):
                                for kx in range(3):
                                    for i in range(2):
                                        nc.tensor.matmul(
                                            ps[:],
                                            w_sb[i][:, ky * 3 + kx, j, :],
                                            x_sb[n][i][
                                                :,
                                                r * RB + ky : r * RB + ky + RB,
                                                kx : kx + W,
                                            ],
                                            start=(idx == 0),
                                            stop=(idx == 17),
                                        )
                                        idx += 1
                            o = opool.tile([128, RB, W], F32, tag="o")
                            nc.scalar.activation(
                                o[:], ps[:],
                                mybir.ActivationFunctionType.Silu,
                                bias=b_sb[j][:], scale=a_sb[j][:],
                            )
                            nc.sync.dma_start(
                                out_d[n, j, :, r * RB : r * RB + RB, :], o[:]
                            )
    if do_split:
        split_sync_waits(nc)
    return nc


def preprocess(x, weight, gamma, beta, running_mean, running_var):
    """Host-side prep: ternarize, fold BN + ternary scale, pad/pack/cast."""
    x = np.asarray(x, dtype=np.float32)
    w = np.asarray(weight, dtype=np.float32)
    gamma = np.asarray(gamma, dtype=np.float32)
    beta = np.asarray(beta, dtype=np.float32)
    rm = np.asarray(running_mean, dtype=np.float32)
    rv = np.asarray(running_var, dtype=np.float32)

    s = np.float32(np.median(np.abs(w)))
    s_c = np.maximum(s, np.float32(1e-5))        # 1/scale of the reference
    scale = np.float32(1.0) / s_c
    t = np.clip(np.round(w * scale), -1.0, 1.0).astype(np.float32)

    inv = gamma / np.sqrt(rv + np.float32(1e-5))
    a = (s_c * inv).astype(np.float32)
    b = (beta - rm * inv).astype(np.float32)

    # [C2, C1, 3, 3] -> [i(c1 chunk), c1in, tap, j(c2 chunk), c2in]
    wp = (
        t.reshape(2, 128, 2, 128, 3, 3)
        .transpose(2, 3, 4, 5, 0, 1)
        .reshape(2, 128, 9, 2, 128)
        .astype(NP_X16)
    )
    ab = np.stack([a.reshape(2, 128), b.reshape(2, 128)], axis=-1).astype(
        np.float32
    )

    xp = np.zeros((B, 2, 128, HP, WP), dtype=NP_X16)
    xp[:, :, :, 1 : H + 1, 1 : W + 1] = x.reshape(B, 2, 128, H, W).astype(NP_X16)
    return xp, wp, ab


_NC_CACHE = {}


def get_nc(repeats=1):
    if repeats not in _NC_CACHE:
        _NC_CACHE[repeats] = build_nc(B_LOC, repeats=repeats)
    return _NC_CACHE[repeats]


def make_in_maps(xp, wp, ab):
    # dim-0 slices of a C-contiguous array are already contiguous
    return [
        {"xp": xp[c * B_LOC : (c + 1) * B_LOC], "wp": wp, "ab": ab}
        for c in range(N_CORES)
    ]


def kernel(x, weight, gamma, beta, running_mean, running_var):
    xp, wp, ab = preprocess(x, weight, gamma, beta, running_mean, running_var)
    nc = get_nc()
    in_maps = make_in_maps(xp, wp, ab)
    # One retry: transient axon-mesh desync / wedged-core errors clear on a
    # fresh attempt (observed repeatedly in this environment).
    try:
        res = run_bass_kernel_spmd(nc, in_maps, list(range(N_CORES)))
    except Exception:
        import time as _time

        _time.sleep(3.0)
        res = run_bass_kernel_spmd(nc, in_maps, list(range(N_CORES)))
    return np.concatenate(
        [r["out"].reshape(B_LOC, C, H, W) for r in res.results], axis=0
    )



# revision 6
# speedup vs baseline: 1.0071x; 1.0071x over previous
"""nn_BitConv: ternary 3x3 conv (stride 1, pad 1) + BatchNorm(eval) + SiLU
on 8 Trainium2 NeuronCores, data-parallel over the batch dimension.

Strategy
--------
Host (numpy, negligible cost): ternarize the weight exactly like the
reference (scale = 1/median|w|, w_q = clamp(round(w*scale))/scale) and keep
only the integer part t in {-1,0,+1} (exact in fp16); fold the 1/scale
factor and the BatchNorm affine into a single per-output-channel
scale/shift (a, b). Pre-transpose the weight into the tensor-engine
stationary layout and zero-pad x to 58x58 / cast to fp16 (same PE rate as
bf16 but 10 mantissa bits: products with ternary weights are exact; only
the fp16 rounding of x contributes error, ~2e-4 relative on the output).

Device (per core, 4 images): the 3x3 conv is 9 shifted matmuls x 2
C1-chunks of K=128 accumulated in PSUM. For each image, C2-chunk (2x128)
and 8-row output block (7 per image), 18 matmuls of [K=128, M=128] x
[128, N=8*56=448] accumulate one PSUM tile; a single ScalarE activation
applies Silu(a*z + b) fused, then the tile is DMA'd out. 1008 back-to-back
matmuls keep the PE warm; ACT/DMA run concurrently. Measured ~180-220 us
per core (burst vs sustained-clock), at the 16-bit PE roofline for the
14.8 GFLOP/core conv.
"""
import numpy as np
import concourse.bass as bass
from concourse import mybir
from concourse.bass_utils import run_bass_kernel_spmd
from concourse.tile import TileContext
from concourse.vector_clock import ScopedClock

X16 = mybir.dt.float16
F32 = mybir.dt.float32
NP_X16 = np.float16

N_CORES = 8
B, C, H, W = 32, 256, 56, 56
B_LOC = B // N_CORES
HP, WP = H + 2, W + 2
RB = 8            # output rows per PSUM tile (N = 8*56 = 448 <= 512)
NRB = H // RB


class _SplitDrainTC(TileContext):
    """This walrus build allows a single sync wait on the SP CTRL (Drain)
    instruction; split the Tile tail drain's waits across extra drains."""

    def _drain_and_barrier(self, tick_clock, wait_clock):
        drain_inst = self.nc.sync.drain()
        wait_clock.add_sem_waits(
            drain_inst.ins, ScopedClock({None: tick_clock.global_clock})
        )
        si = drain_inst.ins.sync_info
        waits = list(si.on_wait or []) if si is not None else []
        if len(waits) > 1:
            si.on_wait = waits[:1]
            for k in range(1, len(waits)):
                d2 = self.nc.sync.drain()
                si2 = d2.ins.sync_info
                if si2 is None:
                    d2.ins.sync_info = mybir.SyncInfo(
                        on_wait=[waits[k]], on_update=[]
                    )
                else:
                    si2.on_wait = [waits[k]]
        self.nc.all_engine_barrier()
        assert self.sems is not None
        popped = self.nc._tile_sem_poison_stack.pop()
        assert popped is self._sem_poison
        self.nc.clear_and_free_semaphores(list(self.sems.allocated().values()))
        self.nc.all_engine_barrier()


def split_sync_waits(nc, limit=1):
    """Hoist excess per-instruction sem waits onto same-engine nops (this
    walrus build allows only `limit` sync waits per instruction)."""
    builders = {
        mybir.EngineType.PE: nc.tensor,
        mybir.EngineType.Activation: nc.scalar,
        mybir.EngineType.DVE: nc.vector,
        mybir.EngineType.Pool: nc.gpsimd,
        mybir.EngineType.SP: nc.sync,
    }
    n_split = 0
    for f in nc.m.functions:
        for bb in f.blocks:
            insts = bb.instructions
            idx = 0
            while idx < len(insts):
                inst = insts[idx]
                si = inst.sync_info
                waits = list(si.on_wait) if (si is not None and si.on_wait) else []
                if len(waits) <= limit:
                    idx += 1
                    continue
                eng = inst.engine
                if eng not in builders:
                    raise RuntimeError(
                        f"split_sync_waits: no builder for engine {eng} "
                        f"on {inst.name} ({type(inst).__name__})"
                    )
                si.on_wait = waits[-limit:]
                carriers = []
                for w in waits[:-limit]:
                    nop = builders[eng].nop(nofuse=True)
                    ci = nop.ins
                    tail_bb = nc.cur_bb.bb
                    assert tail_bb.instructions[-1] is ci
                    tail_bb.instructions.pop()
                    ci.sync_info = mybir.SyncInfo(on_wait=[w], on_update=[])
                    carriers.append(ci)
                for k, ci in enumerate(carriers):
                    insts.insert(idx + k, ci)
                n_split += 1
                idx += len(carriers) + 1
    return n_split


def build_nc(b_loc=B_LOC, repeats=1, do_split=True, hw_loop=False,
             variant="base"):
    nc = bass.Bass()
    xp_d = nc.dram_tensor("xp", [b_loc, 2, 128, HP, WP], X16, kind="ExternalInput")
    wp_d = nc.dram_tensor("wp", [2, 128, 9, 2, 128], X16, kind="ExternalInput")
    ab_d = nc.dram_tensor("ab", [2, 128, 2], F32, kind="ExternalInput")
    out_d = nc.dram_tensor("out", [b_loc, 2, 128, H, W], F32, kind="ExternalOutput")

    with _SplitDrainTC(nc) as tc:
        with (
            tc.tile_pool(name="consts", bufs=1) as consts,
            tc.tile_pool(name="xpool", bufs=1) as xpool,
            tc.tile_pool(name="psum", bufs=8, space="PSUM") as psum,
            tc.tile_pool(name="opool", bufs=4) as opool,
        ):
            w_sb = []
            for i in range(2):
                w = consts.tile([128, 9, 2, 128], X16, tag=f"w{i}")
                nc.sync.dma_start(w[:], wp_d[i])
                w_sb.append(w)
            a_sb, b_sb = [], []
            for j in range(2):
                a = consts.tile([128, 1], F32, tag=f"a{j}")
                nc.sync.dma_start(a[:], ab_d[j, :, 0:1])
                a_sb.append(a)
                bt = consts.tile([128, 1], F32, tag=f"b{j}")
                nc.sync.dma_start(bt[:], ab_d[j, :, 1:2])
                b_sb.append(bt)
            x_sb = [[None] * 2 for _ in range(b_loc)]
            for n in range(b_loc):
                for i in range(2):
                    xt = xpool.tile([128, HP, WP], X16, tag=f"x{n}_{i}")
                    nc.sync.dma_start(xt[:], xp_d[n, i])
                    x_sb[n][i] = xt

            def body():
                if variant == "tap_outer":
                    for n in range(b_loc):
                        for j in range(2):
                            pss = [
                                psum.tile([128, RB, W], F32, tag="ps")
                                for _ in range(NRB)
                            ]
                            for ky in range(3):
                                for kx in range(3):
                                    for i in range(2):
                                        first = ky == 0 and kx == 0 and i == 0
                                        last = ky == 2 and kx == 2 and i == 1
                                        for r in range(NRB):
                                            nc.tensor.matmul(
                                                pss[r][:],
                                                w_sb[i][:, ky * 3 + kx, j, :],
                                                x_sb[n][i][
                                                    :,
                                                    r * RB + ky : r * RB + ky + RB,
                                                    kx : kx + W,
                                                ],
                                                start=first,
                                                stop=last,
                                            )
                            for r in range(NRB):
                                o = opool.tile([128, RB, W], F32, tag="o")
                                nc.scalar.activation(
                                    o[:], pss[r][:],
                                    mybir.ActivationFunctionType.Silu,
                                    bias=b_sb[j][:], scale=a_sb[j][:],
                                )
                                nc.sync.dma_start(
                                    out_d[n, j, :, r * RB : r * RB + RB, :],
                                    o[:],
                                )
                    return
                for n in range(b_loc):
                    for j in range(2):
                        for r in range(NRB):
                            ps = psum.tile([128, RB, W], F32, tag="ps")
                            idx = 0
                            for ky in range(3):
                                for kx in range(3):
                                    for i in range(2):
                                        nc.tensor.matmul(
                                            ps[:],
                                            w_sb[i][:, ky * 3 + kx, j, :],
                                            x_sb[n][i][
                                                :,
                                                r * RB + ky : r * RB + ky + RB,
                                                kx : kx + W,
                                            ],
                                            start=(idx == 0),
                                            stop=(idx == 17),
                                        )
                                        idx += 1
                            if variant == "no_act":
                                continue
                            o = opool.tile([128, RB, W], F32, tag="o")
                            nc.scalar.activation(
                                o[:], ps[:],
                                mybir.ActivationFunctionType.Silu,
                                bias=b_sb[j][:], scale=a_sb[j][:],
                            )
                            nc.sync.dma_start(
                                out_d[n, j, :, r * RB : r * RB + RB, :], o[:]
                            )

            if hw_loop:
                n_iter, n_body = repeats
                with tc.For_i(0, n_iter):
                    for _ in range(n_body):
                        body()
            else:
                for _rep in range(repeats):
                    body()
    if do_split:
        split_sync_waits(nc)
    return nc


def preprocess(x, weight, gamma, beta, running_mean, running_var):
    """Host-side prep: ternarize, fold BN + ternary scale, pad/pack/cast."""
    x = np.asarray(x, dtype=np.float32)
    w = np.asarray(weight, dtype=np.float32)
    gamma = np.asarray(gamma, dtype=np.float32)
    beta = np.asarray(beta, dtype=np.float32)
    rm = np.asarray(running_mean, dtype=np.float32)
    rv = np.asarray(running_var, dtype=np.float32)

    s = np.float32(np.median(np.abs(w)))
    s_c = np.maximum(s, np.float32(1e-5))        # 1/scale of the reference
    scale = np.float32(1.0) / s_c
    t = np.clip(np.round(w * scale), -1.0, 1.0).astype(np.float32)

    inv = gamma / np.sqrt(rv + np.float32(1e-5))
    a = (s_c * inv).astype(np.float32)
    b = (beta - rm * inv).astype(np.float32)

    # [C2, C1, 3, 3] -> [i(c1 chunk), c1in, tap, j(c2 chunk), c2in]
    wp = (
        t.reshape(2, 128, 2, 128, 3, 3)
        .transpose(2, 3, 4, 5, 0, 1)
        .reshape(2, 128, 9, 2, 128)
        .astype(NP_X16)
    )
    ab = np.stack([a.reshape(2, 128), b.reshape(2, 128)], axis=-1).astype(
        np.float32
    )

    xp = np.zeros((B, 2, 128, HP, WP), dtype=NP_X16)
    xp[:, :, :, 1 : H + 1, 1 : W + 1] = x.reshape(B, 2, 128, H, W).astype(NP_X16)
    return xp, wp, ab


_NC_CACHE = {}


def get_nc(repeats=1):
    if repeats not in _NC_CACHE:
        _NC_CACHE[repeats] = build_nc(B_LOC, repeats=repeats)
    return _NC_CACHE[repeats]


def make_in_maps(xp, wp, ab):
    # dim-0 slices of a C-contiguous array are already contiguous
    return [
        {"xp": xp[c * B_LOC : (c + 1) * B_LOC], "wp": wp, "ab": ab}
        for c in range(N_CORES)
    ]


def kernel(x, weight, gamma, beta, running_mean, running_var):
    xp, wp, ab = preprocess(x, weight, gamma, beta, running_mean, running_var)
    nc = get_nc()
    in_maps = make_in_maps(xp, wp, ab)
    # One retry: transient axon-mesh desync / wedged-core errors clear on a
    # fresh attempt (observed repeatedly in this environment).
    try:
        res = run_bass_kernel_spmd(nc, in_maps, list(range(N_CORES)))
    except Exception:
        import time as _time

        _time.sleep(3.0)
        res = run_bass_kernel_spmd(nc, in_maps, list(range(N_CORES)))
    return np.concatenate(
        [r["out"].reshape(B_LOC, C, H, W) for r in res.results], axis=0
    )



# revision 21
# speedup vs baseline: 1.1177x; 1.1098x over previous
"""nn_BitConv: ternary 3x3 conv (stride 1, pad 1) + BatchNorm(eval) + SiLU
on 8 Trainium2 NeuronCores, data-parallel over the batch dimension.

Strategy
--------
Host (numpy, negligible cost): ternarize the weight exactly like the
reference (scale = 1/median|w|, w_q = clamp(round(w*scale))/scale) and keep
only the integer part t in {-1,0,+1} (exact in fp16); fold the 1/scale
factor and the BatchNorm affine into a single per-output-channel
scale/shift (a, b). Pre-transpose the weight into the tensor-engine
stationary layout and zero-pad x to 58x58 / cast to fp16 (same PE rate as
bf16 but 10 mantissa bits: products with ternary weights are exact; only
the fp16 rounding of x contributes error, ~2e-4 relative on the output).

Device (per core, 4 images): the 3x3 conv is 9 shifted matmuls x 2
C1-chunks of K=128 accumulated in PSUM. For each image, C2-chunk (2x128)
and 8-row output block (7 per image), 18 matmuls of [K=128, M=128] x
[128, N=8*56=448] accumulate one PSUM tile; a single ScalarE activation
applies Silu(a*z + b) fused, then the tile is DMA'd out. 1008 back-to-back
matmuls keep the PE warm; ACT/DMA run concurrently. Measured ~180-220 us
per core (burst vs sustained-clock), at the 16-bit PE roofline for the
14.8 GFLOP/core conv.
"""
import numpy as np
import concourse.bass as bass
from concourse import mybir
from concourse.bass_utils import run_bass_kernel_spmd
from concourse.tile import TileContext
from concourse.vector_clock import ScopedClock

X16 = mybir.dt.float16
F32 = mybir.dt.float32
NP_X16 = np.float16

N_CORES = 8
B, C, H, W = 32, 256, 56, 56
B_LOC = B // N_CORES
HP, WP = H + 2, W + 2
RB = 8            # output rows per PSUM tile (N = 8*56 = 448 <= 512)
NRB = H // RB
YB = 14           # winograd: output rows per PSUM tile (N = 14*28 = 392)
NYB = H // YB
TW = W // 2       # winograd F(2,3) output pairs per row


class _SplitDrainTC(TileContext):
    """This walrus build allows a single sync wait on the SP CTRL (Drain)
    instruction; split the Tile tail drain's waits across extra drains."""

    def _drain_and_barrier(self, tick_clock, wait_clock):
        drain_inst = self.nc.sync.drain()
        wait_clock.add_sem_waits(
            drain_inst.ins, ScopedClock({None: tick_clock.global_clock})
        )
        si = drain_inst.ins.sync_info
        waits = list(si.on_wait or []) if si is not None else []
        if len(waits) > 1:
            si.on_wait = waits[:1]
            for k in range(1, len(waits)):
                d2 = self.nc.sync.drain()
                si2 = d2.ins.sync_info
                if si2 is None:
                    d2.ins.sync_info = mybir.SyncInfo(
                        on_wait=[waits[k]], on_update=[]
                    )
                else:
                    si2.on_wait = [waits[k]]
        self.nc.all_engine_barrier()
        assert self.sems is not None
        popped = self.nc._tile_sem_poison_stack.pop()
        assert popped is self._sem_poison
        self.nc.clear_and_free_semaphores(list(self.sems.allocated().values()))
        self.nc.all_engine_barrier()


def split_sync_waits(nc, limit=1):
    """Hoist excess per-instruction sem waits onto same-engine nops (this
    walrus build allows only `limit` sync waits per instruction)."""
    builders = {
        mybir.EngineType.PE: nc.tensor,
        mybir.EngineType.Activation: nc.scalar,
        mybir.EngineType.DVE: nc.vector,
        mybir.EngineType.Pool: nc.gpsimd,
        mybir.EngineType.SP: nc.sync,
    }
    n_split = 0
    for f in nc.m.functions:
        for bb in f.blocks:
            insts = bb.instructions
            idx = 0
            while idx < len(insts):
                inst = insts[idx]
                si = inst.sync_info
                waits = list(si.on_wait) if (si is not None and si.on_wait) else []
                if len(waits) <= limit:
                    idx += 1
                    continue
                eng = inst.engine
                if eng not in builders:
                    raise RuntimeError(
                        f"split_sync_waits: no builder for engine {eng} "
                        f"on {inst.name} ({type(inst).__name__})"
                    )
                si.on_wait = waits[-limit:]
                carriers = []
                for w in waits[:-limit]:
                    nop = builders[eng].nop(nofuse=True)
                    ci = nop.ins
                    tail_bb = nc.cur_bb.bb
                    assert tail_bb.instructions[-1] is ci
                    tail_bb.instructions.pop()
                    ci.sync_info = mybir.SyncInfo(on_wait=[w], on_update=[])
                    carriers.append(ci)
                for k, ci in enumerate(carriers):
                    insts.insert(idx + k, ci)
                n_split += 1
                idx += len(carriers) + 1
    return n_split


def build_wino_nc(b_loc=B_LOC, repeats=1, do_split=True, hw_loop=False,
                  v_engines=("vector", "gpsimd"), skip_inverse=False):
    """1D Winograd F(2,3) along W: 12 matmul streams (4 k-terms x 3 ky) of
    K=256 replace the 18 direct-tap streams -> 2/3 of the PE row traffic.
    V transform (4 shifted add/subs of x) on DVE+Pool; inverse transform
    A^T (2 adds + 2 subs of the four M_k PSUM tiles) on DVE; BN+SiLU on ACT."""
    nc = bass.Bass()
    xp_d = nc.dram_tensor("xp", [b_loc, 2, 128, HP, WP], X16, kind="ExternalInput")
    w2_d = nc.dram_tensor("w2", [2, 128, 4, 3, 2, 128], X16, kind="ExternalInput")
    ab_d = nc.dram_tensor("ab", [2, 128, 2], F32, kind="ExternalInput")
    out_d = nc.dram_tensor("out", [b_loc, 2, 128, H, W], F32, kind="ExternalOutput")

    with _SplitDrainTC(nc) as tc:
        with (
            tc.tile_pool(name="consts", bufs=1) as consts,
            tc.tile_pool(name="xpool", bufs=1) as xpool,
            tc.tile_pool(name="vpool", bufs=2) as vpool,
            tc.tile_pool(name="psum", bufs=8, space="PSUM") as psum,
            tc.tile_pool(name="tpool", bufs=4) as tpool,
            tc.tile_pool(name="ypool", bufs=3) as ypool,
            tc.tile_pool(name="opool", bufs=3) as opool,
        ):
            w_sb = []
            for i in range(2):
                w = consts.tile([128, 4, 3, 2, 128], X16, tag=f"w{i}")
                nc.sync.dma_start(w[:], w2_d[i])
                w_sb.append(w)
            a_sb, b_sb = [], []
            for j in range(2):
                a = consts.tile([128, 1], F32, tag=f"a{j}")
                nc.sync.dma_start(a[:], ab_d[j, :, 0:1])
                a_sb.append(a)
                bt = consts.tile([128, 1], F32, tag=f"b{j}")
                nc.sync.dma_start(bt[:], ab_d[j, :, 1:2])
                b_sb.append(bt)
            x_sb = [[None] * 2 for _ in range(b_loc)]
            for n in range(b_loc):
                for i in range(2):
                    xt = xpool.tile([128, HP, WP], X16, tag=f"x{n}_{i}")
                    nc.sync.dma_start(xt[:], xp_d[n, i])
                    x_sb[n][i] = xt

            def make_v(n):
                """V[k, y, tw] combos; DVE does i=0 plane, Pool does i=1."""
                vs = []
                for i in range(2):
                    eng = getattr(nc, v_engines[i])
                    v = vpool.tile([128, 4, HP, TW], X16, tag=f"v{i}",
                                   name=f"v_{n}_{i}")
                    xs = x_sb[n][i]

                    def sl(c0):
                        return xs[:, :, c0 : c0 + 2 * TW - 1 : 2]

                    eng.tensor_sub(v[:, 0], sl(0), sl(2))
                    eng.tensor_add(v[:, 1], sl(1), sl(2))
                    eng.tensor_sub(v[:, 2], sl(2), sl(1))
                    eng.tensor_sub(v[:, 3], sl(1), sl(3))
                    vs.append(v)
                return vs

            def mm_block(n, v_n):
                for j in range(2):
                    for blk in range(NYB):
                        pss = []
                        for k in range(4):
                            ps = psum.tile([128, YB, TW], F32, tag="ps",
                                           name=f"ps_{n}_{j}_{blk}_{k}")
                            for ky in range(3):
                                for i in range(2):
                                    nc.tensor.matmul(
                                        ps[:],
                                        w_sb[i][:, k, ky, j, :],
                                        v_n[i][
                                            :, k,
                                            blk * YB + ky : blk * YB + ky + YB,
                                            :,
                                        ],
                                        start=(ky == 0 and i == 0),
                                        stop=(ky == 2 and i == 1),
                                    )
                            pss.append(ps)
                        if skip_inverse:
                            continue
                        # ACT drains all four M_k tiles to SBUF right away
                        # (frees the PSUM banks without waiting on DVE);
                        # DVE computes the A^T combos entirely from SBUF.
                        ms = []
                        for k in range(4):
                            m = tpool.tile([128, YB, TW], F32, tag=f"m{k}",
                                           name=f"m_{n}_{j}_{blk}_{k}")
                            nc.scalar.copy(m[:], pss[k][:])
                            ms.append(m)
                        te = tpool.tile([128, YB, TW], F32, tag="te")
                        to = tpool.tile([128, YB, TW], F32, tag="to")
                        y = ypool.tile([128, YB, W], F32, tag="y")
                        nc.vector.tensor_add(te[:], ms[0][:], ms[1][:])
                        nc.vector.tensor_sub(to[:], ms[1][:], ms[2][:])
                        nc.vector.tensor_add(y[:, :, 0::2], te[:], ms[2][:])
                        nc.vector.tensor_sub(y[:, :, 1::2], to[:], ms[3][:])
                        oo = opool.tile([128, YB, W], F32, tag="o")
                        nc.scalar.activation(
                            oo[:], y[:],
                            mybir.ActivationFunctionType.Silu,
                            bias=b_sb[j][:], scale=a_sb[j][:],
                        )
                        nc.sync.dma_start(
                            out_d[n, j, :, blk * YB : blk * YB + YB, :], oo[:]
                        )

            def body():
                v_tiles = [None] * b_loc
                v_tiles[0] = make_v(0)
                for n in range(b_loc):
                    if n + 1 < b_loc:
                        v_tiles[n + 1] = make_v(n + 1)
                    mm_block(n, v_tiles[n])

            if hw_loop:
                n_iter, n_body = repeats
                with tc.For_i(0, n_iter):
                    for _ in range(n_body):
                        body()
            else:
                for _rep in range(repeats):
                    body()
    if do_split:
        split_sync_waits(nc)
    return nc


def build_nc(b_loc=B_LOC, repeats=1, do_split=True, hw_loop=False,
             variant="base"):
    if variant == "wino":
        return build_wino_nc(b_loc, repeats, do_split, hw_loop)
    if variant == "wino_dve":
        return build_wino_nc(b_loc, repeats, do_split, hw_loop,
                             v_engines=("vector", "vector"))
    if variant == "wino_gps":
        return build_wino_nc(b_loc, repeats, do_split, hw_loop,
                             v_engines=("gpsimd", "gpsimd"))
    if variant == "wino_noinv":
        return build_wino_nc(b_loc, repeats, do_split, hw_loop,
                             v_engines=("vector", "vector"),
                             skip_inverse=True)
    nc = bass.Bass()
    xp_d = nc.dram_tensor("xp", [b_loc, 2, 128, HP, WP], X16, kind="ExternalInput")
    wp_d = nc.dram_tensor("wp", [2, 128, 9, 2, 128], X16, kind="ExternalInput")
    ab_d = nc.dram_tensor("ab", [2, 128, 2], F32, kind="ExternalInput")
    out_d = nc.dram_tensor("out", [b_loc, 2, 128, H, W], F32, kind="ExternalOutput")

    with _SplitDrainTC(nc) as tc:
        with (
            tc.tile_pool(name="consts", bufs=1) as consts,
            tc.tile_pool(name="xpool", bufs=1) as xpool,
            tc.tile_pool(name="psum", bufs=8, space="PSUM") as psum,
            tc.tile_pool(name="opool", bufs=4) as opool,
        ):
            w_sb = []
            for i in range(2):
                w = consts.tile([128, 9, 2, 128], X16, tag=f"w{i}")
                nc.sync.dma_start(w[:], wp_d[i])
                w_sb.append(w)
            a_sb, b_sb = [], []
            for j in range(2):
                a = consts.tile([128, 1], F32, tag=f"a{j}")
                nc.sync.dma_start(a[:], ab_d[j, :, 0:1])
                a_sb.append(a)
                bt = consts.tile([128, 1], F32, tag=f"b{j}")
                nc.sync.dma_start(bt[:], ab_d[j, :, 1:2])
                b_sb.append(bt)
            x_sb = [[None] * 2 for _ in range(b_loc)]
            for n in range(b_loc):
                for i in range(2):
                    xt = xpool.tile([128, HP, WP], X16, tag=f"x{n}_{i}")
                    nc.sync.dma_start(xt[:], xp_d[n, i])
                    x_sb[n][i] = xt

            def body():
                if variant in ("small_n", "tiny_n"):
                    rb = 4 if variant == "small_n" else 2
                    nrb = H // rb
                    for n in range(b_loc):
                        for j in range(2):
                            for r in range(nrb):
                                ps = psum.tile([128, rb, W], F32, tag="ps")
                                idx = 0
                                for ky in range(3):
                                    for kx in range(3):
                                        for i in range(2):
                                            nc.tensor.matmul(
                                                ps[:],
                                                w_sb[i][:, ky * 3 + kx, j, :],
                                                x_sb[n][i][
                                                    :,
                                                    r * rb + ky : r * rb + ky + rb,
                                                    kx : kx + W,
                                                ],
                                                start=(idx == 0),
                                                stop=(idx == 17),
                                            )
                                            idx += 1
                                o = opool.tile([128, rb, W], F32, tag="o")
                                nc.scalar.activation(
                                    o[:], ps[:],
                                    mybir.ActivationFunctionType.Silu,
                                    bias=b_sb[j][:], scale=a_sb[j][:],
                                )
                                nc.sync.dma_start(
                                    out_d[n, j, :, r * rb : r * rb + rb, :],
                                    o[:],
                                )
                    return
                if variant == "ldw":
                    # explicit weight load once per (tap, i); matmuls flagged
                    # ldweights=False reuse the loaded stationary operand
                    for n in range(b_loc):
                        for j in range(2):
                            pss = [
                                psum.tile([128, RB, W], F32, tag="ps",
                                          name=f"psl_{n}_{j}_{r}")
                                for r in range(NRB)
                            ]
                            for ky in range(3):
                                for kx in range(3):
                                    for i in range(2):
                                        first = ky == 0 and kx == 0 and i == 0
                                        last = ky == 2 and kx == 2 and i == 1
                                        wap = w_sb[i][:, ky * 3 + kx, j, :]
                                        nc.tensor.ldweights(wap)
                                        for r in range(NRB):
                                            h = nc.tensor.matmul(
                                                pss[r][:],
                                                wap,
                                                x_sb[n][i][
                                                    :,
                                                    r * RB + ky : r * RB + ky + RB,
                                                    kx : kx + W,
                                                ],
                                                start=first,
                                                stop=last,
                                            )
                                            h.ins.ldweights = False
                            for r in range(NRB):
                                o = opool.tile([128, RB, W], F32, tag="o")
                                nc.scalar.activation(
                                    o[:], pss[r][:],
                                    mybir.ActivationFunctionType.Silu,
                                    bias=b_sb[j][:], scale=a_sb[j][:],
                                )
                                nc.sync.dma_start(
                                    out_d[n, j, :, r * RB : r * RB + RB, :],
                                    o[:],
                                )
                    return
                if variant == "tap_outer":
                    for n in range(b_loc):
                        for j in range(2):
                            pss = [
                                psum.tile([128, RB, W], F32, tag="ps",
                                          name=f"ps_{n}_{j}_{r}")
                                for r in range(NRB)
                            ]
                            for ky in range(3):
                                for kx in range(3):
                                    for i in range(2):
                                        first = ky == 0 and kx == 0 and i == 0
                                        last = ky == 2 and kx == 2 and i == 1
                                        for r in range(NRB):
                                            nc.tensor.matmul(
                                                pss[r][:],
                                                w_sb[i][:, ky * 3 + kx, j, :],
                                                x_sb[n][i][
                                                    :,
                                                    r * RB + ky : r * RB + ky + RB,
                                                    kx : kx + W,
                                                ],
                                                start=first,
                                                stop=last,
                                            )
                            for r in range(NRB):
                                o = opool.tile([128, RB, W], F32, tag="o")
                                nc.scalar.activation(
                                    o[:], pss[r][:],
                                    mybir.ActivationFunctionType.Silu,
                                    bias=b_sb[j][:], scale=a_sb[j][:],
                                )
                                nc.sync.dma_start(
                                    out_d[n, j, :, r * RB : r * RB + RB, :],
                                    o[:],
                                )
                    return
                for n in range(b_loc):
                    for j in range(2):
                        for r in range(NRB):
                            ps = psum.tile([128, RB, W], F32, tag="ps")
                            idx = 0
                            for ky in range(3):
                                for kx in range(3):
                                    for i in range(2):
                                        nc.tensor.matmul(
                                            ps[:],
                                            w_sb[i][:, ky * 3 + kx, j, :],
                                            x_sb[n][i][
                                                :,
                                                r * RB + ky : r * RB + ky + RB,
                                                kx : kx + W,
                                            ],
                                            start=(idx == 0),
                                            stop=(idx == 17),
                                        )
                                        idx += 1
                            if variant == "no_act":
                                continue
                            o = opool.tile([128, RB, W], F32, tag="o")
                            nc.scalar.activation(
                                o[:], ps[:],
                                mybir.ActivationFunctionType.Silu,
                                bias=b_sb[j][:], scale=a_sb[j][:],
                            )
                            nc.sync.dma_start(
                                out_d[n, j, :, r * RB : r * RB + RB, :], o[:]
                            )

            if hw_loop:
                n_iter, n_body = repeats
                with tc.For_i(0, n_iter):
                    for _ in range(n_body):
                        body()
            else:
                for _rep in range(repeats):
                    body()
    if variant == "strip_ldw":
        # timing probe only: remove every InstLdweights (numerics garbage)
        for f in nc.m.functions:
            for bb in f.blocks:
                keep = []
                pending_waits = []
                for inst in bb.instructions:
                    if type(inst).__name__ == "InstLdweights":
                        si = inst.sync_info
                        if si and si.on_wait:
                            pending_waits.extend(si.on_wait)
                        continue
                    if pending_waits:
                        si = inst.sync_info
                        if si is None:
                            inst.sync_info = mybir.SyncInfo(
                                on_wait=pending_waits, on_update=[]
                            )
                        else:
                            si.on_wait = list(si.on_wait) + pending_waits
                        pending_waits = []
                    keep.append(inst)
                bb.instructions[:] = keep
    if do_split:
        split_sync_waits(nc)
    return nc


def preprocess(x, weight, gamma, beta, running_mean, running_var):
    """Host-side prep: ternarize, fold BN + ternary scale, pad/pack/cast."""
    x = np.asarray(x, dtype=np.float32)
    w = np.asarray(weight, dtype=np.float32)
    gamma = np.asarray(gamma, dtype=np.float32)
    beta = np.asarray(beta, dtype=np.float32)
    rm = np.asarray(running_mean, dtype=np.float32)
    rv = np.asarray(running_var, dtype=np.float32)

    s = np.float32(np.median(np.abs(w)))
    s_c = np.maximum(s, np.float32(1e-5))        # 1/scale of the reference
    scale = np.float32(1.0) / s_c
    t = np.clip(np.round(w * scale), -1.0, 1.0).astype(np.float32)

    inv = gamma / np.sqrt(rv + np.float32(1e-5))
    a = (s_c * inv).astype(np.float32)
    b = (beta - rm * inv).astype(np.float32)

    # [C2, C1, 3, 3] -> [i(c1 chunk), c1in, tap, j(c2 chunk), c2in]
    wp = (
        t.reshape(2, 128, 2, 128, 3, 3)
        .transpose(2, 3, 4, 5, 0, 1)
        .reshape(2, 128, 9, 2, 128)
        .astype(NP_X16)
    )
    ab = np.stack([a.reshape(2, 128), b.reshape(2, 128)], axis=-1).astype(
        np.float32
    )

    xp = np.zeros((B, 2, 128, HP, WP), dtype=NP_X16)
    xp[:, :, :, 1 : H + 1, 1 : W + 1] = x.reshape(B, 2, 128, H, W).astype(NP_X16)

    # winograd F(2,3) weights: G @ w over the kx dim; halves of ternary
    # sums -> exact in fp16.  gw[k, ky, c2, c1]
    gw = np.empty((4, 3) + t.shape[:2], dtype=np.float32)
    for ky in range(3):
        g = t[:, :, ky, :]
        gw[0, ky] = g[..., 0]
        gw[1, ky] = (g[..., 0] + g[..., 1] + g[..., 2]) * 0.5
        gw[2, ky] = (g[..., 0] - g[..., 1] + g[..., 2]) * 0.5
        gw[3, ky] = g[..., 2]
    # -> [i, c1in, k, ky, j, c2in]
    w2 = (
        gw.reshape(4, 3, 2, 128, 2, 128)
        .transpose(4, 5, 0, 1, 2, 3)
        .astype(NP_X16)
    )
    return {"xp": xp, "wp": wp, "ab": ab, "w2": np.ascontiguousarray(w2)}


_NC_CACHE = {}

PROD_VARIANT = "wino_dve"


def get_nc(repeats=1, hw_loop=False, variant=None):
    v = PROD_VARIANT if variant is None else variant
    key = (repeats if not hw_loop else ("hw",) + tuple(repeats), v)
    if key not in _NC_CACHE:
        _NC_CACHE[key] = build_nc(B_LOC, repeats=repeats, hw_loop=hw_loop,
                                  variant=v)
    return _NC_CACHE[key]


def make_in_maps(prep):
    # dim-0 slices of a C-contiguous array are already contiguous
    xp = prep["xp"]
    rest = {k: v for k, v in prep.items() if k != "xp"}
    return [
        {"xp": xp[c * B_LOC : (c + 1) * B_LOC], **rest}
        for c in range(N_CORES)
    ]


def kernel(x, weight, gamma, beta, running_mean, running_var):
    prep = preprocess(x, weight, gamma, beta, running_mean, running_var)
    nc = get_nc()
    in_maps = make_in_maps(prep)
    # One retry: transient axon-mesh desync / wedged-core errors clear on a
    # fresh attempt (observed repeatedly in this environment).
    try:
        res = run_bass_kernel_spmd(nc, in_maps, list(range(N_CORES)))
    except Exception:
        import time as _time

        _time.sleep(3.0)
        res = run_bass_kernel_spmd(nc, in_maps, list(range(N_CORES)))
    return np.concatenate(
        [r["out"].reshape(B_LOC, C, H, W) for r in res.results], axis=0
    )



# revision 26
# speedup vs baseline: 1.2619x; 1.1290x over previous
"""nn_BitConv: ternary 3x3 conv (stride 1, pad 1) + BatchNorm(eval) + SiLU
on 8 Trainium2 NeuronCores, data-parallel over the batch dimension.

Strategy
--------
Host (numpy, negligible cost): ternarize the weight exactly like the
reference (scale = 1/median|w|, w_q = clamp(round(w*scale))/scale) and keep
only the integer part t in {-1,0,+1} (exact in fp16); fold the 1/scale
factor and the BatchNorm affine into a single per-output-channel
scale/shift (a, b). Pre-transpose the weight into the tensor-engine
stationary layout and zero-pad x to 58x58 / cast to fp16 (same PE rate as
bf16 but 10 mantissa bits: products with ternary weights are exact; only
the fp16 rounding of x contributes error, ~2e-4 relative on the output).

Device (per core, 4 images): the 3x3 conv is 9 shifted matmuls x 2
C1-chunks of K=128 accumulated in PSUM. For each image, C2-chunk (2x128)
and 8-row output block (7 per image), 18 matmuls of [K=128, M=128] x
[128, N=8*56=448] accumulate one PSUM tile; a single ScalarE activation
applies Silu(a*z + b) fused, then the tile is DMA'd out. 1008 back-to-back
matmuls keep the PE warm; ACT/DMA run concurrently. Measured ~180-220 us
per core (burst vs sustained-clock), at the 16-bit PE roofline for the
14.8 GFLOP/core conv.
"""
import numpy as np
from ml_dtypes import float8_e4m3fn
import concourse.bass as bass
from concourse import mybir
from concourse.bass_utils import run_bass_kernel_spmd
from concourse.tile import TileContext
from concourse.vector_clock import ScopedClock

X16 = mybir.dt.float16
F32 = mybir.dt.float32
NP_X16 = np.float16

N_CORES = 8
B, C, H, W = 32, 256, 56, 56
B_LOC = B // N_CORES
HP, WP = H + 2, W + 2
RB = 8            # output rows per PSUM tile (N = 8*56 = 448 <= 512)
NRB = H // RB
YB = 14           # winograd: output rows per PSUM tile (N = 14*28 = 392)
NYB = H // YB
TW = W // 2       # winograd F(2,3) output pairs per row


class _SplitDrainTC(TileContext):
    """This walrus build allows a single sync wait on the SP CTRL (Drain)
    instruction; split the Tile tail drain's waits across extra drains."""

    def _drain_and_barrier(self, tick_clock, wait_clock):
        drain_inst = self.nc.sync.drain()
        wait_clock.add_sem_waits(
            drain_inst.ins, ScopedClock({None: tick_clock.global_clock})
        )
        si = drain_inst.ins.sync_info
        waits = list(si.on_wait or []) if si is not None else []
        if len(waits) > 1:
            si.on_wait = waits[:1]
            for k in range(1, len(waits)):
                d2 = self.nc.sync.drain()
                si2 = d2.ins.sync_info
                if si2 is None:
                    d2.ins.sync_info = mybir.SyncInfo(
                        on_wait=[waits[k]], on_update=[]
                    )
                else:
                    si2.on_wait = [waits[k]]
        self.nc.all_engine_barrier()
        assert self.sems is not None
        popped = self.nc._tile_sem_poison_stack.pop()
        assert popped is self._sem_poison
        self.nc.clear_and_free_semaphores(list(self.sems.allocated().values()))
        self.nc.all_engine_barrier()


def split_sync_waits(nc, limit=1):
    """Hoist excess per-instruction sem waits onto same-engine nops (this
    walrus build allows only `limit` sync waits per instruction)."""
    builders = {
        mybir.EngineType.PE: nc.tensor,
        mybir.EngineType.Activation: nc.scalar,
        mybir.EngineType.DVE: nc.vector,
        mybir.EngineType.Pool: nc.gpsimd,
        mybir.EngineType.SP: nc.sync,
    }
    n_split = 0
    for f in nc.m.functions:
        for bb in f.blocks:
            insts = bb.instructions
            idx = 0
            while idx < len(insts):
                inst = insts[idx]
                si = inst.sync_info
                waits = list(si.on_wait) if (si is not None and si.on_wait) else []
                if len(waits) <= limit:
                    idx += 1
                    continue
                eng = inst.engine
                if eng not in builders:
                    raise RuntimeError(
                        f"split_sync_waits: no builder for engine {eng} "
                        f"on {inst.name} ({type(inst).__name__})"
                    )
                si.on_wait = waits[-limit:]
                carriers = []
                for w in waits[:-limit]:
                    nop = builders[eng].nop(nofuse=True)
                    ci = nop.ins
                    tail_bb = nc.cur_bb.bb
                    assert tail_bb.instructions[-1] is ci
                    tail_bb.instructions.pop()
                    ci.sync_info = mybir.SyncInfo(on_wait=[w], on_update=[])
                    carriers.append(ci)
                for k, ci in enumerate(carriers):
                    insts.insert(idx + k, ci)
                n_split += 1
                idx += len(carriers) + 1
    return n_split


def build_wino_nc(b_loc=B_LOC, repeats=1, do_split=True, hw_loop=False,
                  v_engines=("vector", "gpsimd"), skip_inverse=False,
                  fp8_k0=False, out16=False, fp8_extra=()):
    """1D Winograd F(2,3) along W: 12 matmul streams (4 k-terms x 3 ky) of
    K=256 replace the 18 direct-tap streams -> 2/3 of the PE row traffic.
    V transform (4 shifted add/subs of x) on DVE+Pool; inverse transform
    A^T (2 adds + 2 subs of the four M_k PSUM tiles) on DVE; BN+SiLU on ACT."""
    nc = bass.Bass()
    xp_d = nc.dram_tensor("xp", [b_loc, 2, 128, HP, WP], X16, kind="ExternalInput")
    w2_d = nc.dram_tensor("w2", [2, 128, 4, 3, 2, 128], X16, kind="ExternalInput")
    ab_d = nc.dram_tensor("ab", [2, 128, 2], F32, kind="ExternalInput")
    F8 = mybir.dt.float8e4
    out_d = nc.dram_tensor("out", [b_loc, 2, 128, H, W],
                           X16 if out16 else F32, kind="ExternalOutput")
    TWP = 32  # fp8 V plane: tw padded to 32 so the DoubleRow ktile stride
    #           (HP*TWP elems) is 16-byte aligned
    if fp8_k0:
        w8_d = nc.dram_tensor("w8", [128, 3, 2, 2, 128], F8,
                              kind="ExternalInput")
    if fp8_extra:
        w8b_d = nc.dram_tensor("w8b", [128, 2, 2, 128], F8,
                               kind="ExternalInput")

    with _SplitDrainTC(nc) as tc:
        with (
            tc.tile_pool(name="consts", bufs=1) as consts,
            tc.tile_pool(name="xpool", bufs=1) as xpool,
            tc.tile_pool(name="vpool", bufs=2) as vpool,
            tc.tile_pool(name="psum", bufs=8, space="PSUM") as psum,
            tc.tile_pool(name="tpool", bufs=4) as tpool,
            tc.tile_pool(name="ypool", bufs=3) as ypool,
            tc.tile_pool(name="opool", bufs=3) as opool,
        ):
            w_sb = []
            for i in range(2):
                w = consts.tile([128, 4, 3, 2, 128], X16, tag=f"w{i}")
                nc.sync.dma_start(w[:], w2_d[i])
                w_sb.append(w)
            if fp8_k0:
                w8_sb = consts.tile([128, 3, 2, 2, 128], F8, tag="w8")
                nc.sync.dma_start(w8_sb[:], w8_d[:])
            if fp8_extra:
                w8b_sb = consts.tile([128, 2, 2, 128], F8, tag="w8b")
                nc.sync.dma_start(w8b_sb[:], w8b_d[:])
            a_sb, b_sb = [], []
            for j in range(2):
                a = consts.tile([128, 1], F32, tag=f"a{j}")
                nc.sync.dma_start(a[:], ab_d[j, :, 0:1])
                a_sb.append(a)
                bt = consts.tile([128, 1], F32, tag=f"b{j}")
                nc.sync.dma_start(bt[:], ab_d[j, :, 1:2])
                b_sb.append(bt)
            x_sb = [[None] * 2 for _ in range(b_loc)]
            for n in range(b_loc):
                for i in range(2):
                    xt = xpool.tile([128, HP, WP], X16, tag=f"x{n}_{i}")
                    nc.sync.dma_start(xt[:], xp_d[n, i])
                    x_sb[n][i] = xt

            def make_v(n):
                """V[k, y, tw] combos; DVE does i=0 plane, Pool does i=1.
                With fp8_k0, the k=0 plane goes to a shared fp8 tile
                [128, 2(i), HP, TWP] consumed by DoubleRow matmuls."""
                vs = []
                v8 = None
                v8b = None
                if fp8_k0:
                    v8 = vpool.tile([128, 2, HP, TWP], F8, tag="v8",
                                    name=f"v8_{n}")
                if fp8_extra:
                    v8b = vpool.tile([128, 2, HP, TWP], F8, tag="v8b",
                                     name=f"v8b_{n}")
                for i in range(2):
                    eng = getattr(nc, v_engines[i])
                    v = vpool.tile([128, 4 - (1 if fp8_k0 else 0), HP, TW],
                                   X16, tag=f"v{i}", name=f"v_{n}_{i}")
                    xs = x_sb[n][i]

                    def sl(c0):
                        return xs[:, :, c0 : c0 + 2 * TW - 1 : 2]

                    if fp8_k0:
                        eng.tensor_sub(v8[:, i, :, 0:TW], sl(0), sl(2))
                        eng.tensor_add(v[:, 0], sl(1), sl(2))
                        eng.tensor_sub(v[:, 1], sl(2), sl(1))
                        eng.tensor_sub(v[:, 2], sl(1), sl(3))
                        if fp8_extra:
                            eng.tensor_sub(v8b[:, i, :, 0:TW], sl(1), sl(3))
                    else:
                        eng.tensor_sub(v[:, 0], sl(0), sl(2))
                        eng.tensor_add(v[:, 1], sl(1), sl(2))
                        eng.tensor_sub(v[:, 2], sl(2), sl(1))
                        eng.tensor_sub(v[:, 3], sl(1), sl(3))
                    vs.append(v)
                return (vs, v8, v8b)

            def mm_block(n, v_all):
                v_n, v8_n, v8b_n = v_all
                for j in range(2):
                    for blk in range(NYB):
                        pss = []
                        if fp8_k0:
                            ps = psum.tile([128, YB, TW], F32, tag="ps",
                                           name=f"ps8_{n}_{j}_{blk}")
                            for ky in range(3):
                                r0 = blk * YB + ky
                                nc.tensor.matmul(
                                    ps[:],
                                    w8_sb[:, ky, j, :, :],
                                    v8_n[:, :, r0 : r0 + YB, 0:TW],
                                    perf_mode=mybir.MatmulPerfMode.DoubleRow,
                                    start=(ky == 0),
                                    stop=(ky == 2),
                                )
                            pss.append(ps)
                        k_lo = 1 if fp8_k0 else 0
                        for k in range(k_lo, 4):
                            ps = psum.tile([128, YB, TW], F32, tag="ps",
                                           name=f"ps_{n}_{j}_{blk}_{k}")
                            fp8_kys = (
                                (0,) if (fp8_extra and k == 3) else ()
                            )
                            for ky in fp8_kys:
                                r0 = blk * YB + ky
                                nc.tensor.matmul(
                                    ps[:],
                                    w8b_sb[:, j, :, :],
                                    v8b_n[:, :, r0 : r0 + YB, 0:TW],
                                    perf_mode=mybir.MatmulPerfMode.DoubleRow,
                                    start=(ky == 0),
                                    stop=False,
                                )
                            for ky in range(3):
                                if ky in fp8_kys:
                                    continue
                                for i in range(2):
                                    nc.tensor.matmul(
                                        ps[:],
                                        w_sb[i][:, k, ky, j, :],
                                        v_n[i][
                                            :, k - k_lo,
                                            blk * YB + ky : blk * YB + ky + YB,
                                            :,
                                        ],
                                        start=(ky == 0 and i == 0
                                               and not fp8_kys),
                                        stop=(ky == 2 and i == 1),
                                    )
                            pss.append(ps)
                        if skip_inverse:
                            continue
                        # ACT drains all four M_k tiles to SBUF right away
                        # (frees the PSUM banks without waiting on DVE);
                        # DVE computes the A^T combos entirely from SBUF.
                        ms = []
                        for k in range(4):
                            m = tpool.tile([128, YB, TW], F32, tag=f"m{k}",
                                           name=f"m_{n}_{j}_{blk}_{k}")
                            nc.scalar.copy(m[:], pss[k][:])
                            ms.append(m)
                        te = tpool.tile([128, YB, TW], F32, tag="te")
                        to = tpool.tile([128, YB, TW], F32, tag="to")
                        y = ypool.tile([128, YB, W], F32, tag="y")
                        nc.vector.tensor_add(te[:], ms[0][:], ms[1][:])
                        nc.vector.tensor_sub(to[:], ms[1][:], ms[2][:])
                        nc.vector.tensor_add(y[:, :, 0::2], te[:], ms[2][:])
                        nc.vector.tensor_sub(y[:, :, 1::2], to[:], ms[3][:])
                        oo = opool.tile([128, YB, W],
                                        X16 if out16 else F32, tag="o")
                        nc.scalar.activation(
                            oo[:], y[:],
                            mybir.ActivationFunctionType.Silu,
                            bias=b_sb[j][:], scale=a_sb[j][:],
                        )
                        nc.sync.dma_start(
                            out_d[n, j, :, blk * YB : blk * YB + YB, :], oo[:]
                        )

            def body():
                v_tiles = [None] * b_loc
                v_tiles[0] = make_v(0)
                for n in range(b_loc):
                    if n + 1 < b_loc:
                        v_tiles[n + 1] = make_v(n + 1)
                    mm_block(n, v_tiles[n])

            if hw_loop:
                n_iter, n_body = repeats
                with tc.For_i(0, n_iter):
                    for _ in range(n_body):
                        body()
            else:
                for _rep in range(repeats):
                    body()
    if do_split:
        split_sync_waits(nc)
    return nc


def build_nc(b_loc=B_LOC, repeats=1, do_split=True, hw_loop=False,
             variant="base"):
    if variant == "wino":
        return build_wino_nc(b_loc, repeats, do_split, hw_loop)
    if variant == "wino_dve":
        return build_wino_nc(b_loc, repeats, do_split, hw_loop,
                             v_engines=("vector", "vector"))
    if variant == "wino_gps":
        return build_wino_nc(b_loc, repeats, do_split, hw_loop,
                             v_engines=("gpsimd", "gpsimd"))
    if variant == "wino_noinv":
        return build_wino_nc(b_loc, repeats, do_split, hw_loop,
                             v_engines=("vector", "vector"),
                             skip_inverse=True)
    if variant == "wino_f8":
        return build_wino_nc(b_loc, repeats, do_split, hw_loop,
                             v_engines=("vector", "vector"), fp8_k0=True)
    if variant == "wino_f8_o16":
        return build_wino_nc(b_loc, repeats, do_split, hw_loop,
                             v_engines=("vector", "vector"), fp8_k0=True,
                             out16=True)
    if variant == "wino_f8x":
        return build_wino_nc(b_loc, repeats, do_split, hw_loop,
                             v_engines=("vector", "vector"), fp8_k0=True,
                             fp8_extra=((3, 0),))
    nc = bass.Bass()
    xp_d = nc.dram_tensor("xp", [b_loc, 2, 128, HP, WP], X16, kind="ExternalInput")
    wp_d = nc.dram_tensor("wp", [2, 128, 9, 2, 128], X16, kind="ExternalInput")
    ab_d = nc.dram_tensor("ab", [2, 128, 2], F32, kind="ExternalInput")
    out_d = nc.dram_tensor("out", [b_loc, 2, 128, H, W], F32, kind="ExternalOutput")

    with _SplitDrainTC(nc) as tc:
        with (
            tc.tile_pool(name="consts", bufs=1) as consts,
            tc.tile_pool(name="xpool", bufs=1) as xpool,
            tc.tile_pool(name="psum", bufs=8, space="PSUM") as psum,
            tc.tile_pool(name="opool", bufs=4) as opool,
        ):
            w_sb = []
            for i in range(2):
                w = consts.tile([128, 9, 2, 128], X16, tag=f"w{i}")
                nc.sync.dma_start(w[:], wp_d[i])
                w_sb.append(w)
            a_sb, b_sb = [], []
            for j in range(2):
                a = consts.tile([128, 1], F32, tag=f"a{j}")
                nc.sync.dma_start(a[:], ab_d[j, :, 0:1])
                a_sb.append(a)
                bt = consts.tile([128, 1], F32, tag=f"b{j}")
                nc.sync.dma_start(bt[:], ab_d[j, :, 1:2])
                b_sb.append(bt)
            x_sb = [[None] * 2 for _ in range(b_loc)]
            for n in range(b_loc):
                for i in range(2):
                    xt = xpool.tile([128, HP, WP], X16, tag=f"x{n}_{i}")
                    nc.sync.dma_start(xt[:], xp_d[n, i])
                    x_sb[n][i] = xt

            def body():
                if variant in ("small_n", "tiny_n"):
                    rb = 4 if variant == "small_n" else 2
                    nrb = H // rb
                    for n in range(b_loc):
                        for j in range(2):
                            for r in range(nrb):
                                ps = psum.tile([128, rb, W], F32, tag="ps")
                                idx = 0
                                for ky in range(3):
                                    for kx in range(3):
                                        for i in range(2):
                                            nc.tensor.matmul(
                                                ps[:],
                                                w_sb[i][:, ky * 3 + kx, j, :],
                                                x_sb[n][i][
                                                    :,
                                                    r * rb + ky : r * rb + ky + rb,
                                                    kx : kx + W,
                                                ],
                                                start=(idx == 0),
                                                stop=(idx == 17),
                                            )
                                            idx += 1
                                o = opool.tile([128, rb, W], F32, tag="o")
                                nc.scalar.activation(
                                    o[:], ps[:],
                                    mybir.ActivationFunctionType.Silu,
                                    bias=b_sb[j][:], scale=a_sb[j][:],
                                )
                                nc.sync.dma_start(
                                    out_d[n, j, :, r * rb : r * rb + rb, :],
                                    o[:],
                                )
                    return
                if variant == "ldw":
                    # explicit weight load once per (tap, i); matmuls flagged
                    # ldweights=False reuse the loaded stationary operand
                    for n in range(b_loc):
                        for j in range(2):
                            pss = [
                                psum.tile([128, RB, W], F32, tag="ps",
                                          name=f"psl_{n}_{j}_{r}")
                                for r in range(NRB)
                            ]
                            for ky in range(3):
                                for kx in range(3):
                                    for i in range(2):
                                        first = ky == 0 and kx == 0 and i == 0
                                        last = ky == 2 and kx == 2 and i == 1
                                        wap = w_sb[i][:, ky * 3 + kx, j, :]
                                        nc.tensor.ldweights(wap)
                                        for r in range(NRB):
                                            h = nc.tensor.matmul(
                                                pss[r][:],
                                                wap,
                                                x_sb[n][i][
                                                    :,
                                                    r * RB + ky : r * RB + ky + RB,
                                                    kx : kx + W,
                                                ],
                                                start=first,
                                                stop=last,
                                            )
                                            h.ins.ldweights = False
                            for r in range(NRB):
                                o = opool.tile([128, RB, W], F32, tag="o")
                                nc.scalar.activation(
                                    o[:], pss[r][:],
                                    mybir.ActivationFunctionType.Silu,
                                    bias=b_sb[j][:], scale=a_sb[j][:],
                                )
                                nc.sync.dma_start(
                                    out_d[n, j, :, r * RB : r * RB + RB, :],
                                    o[:],
                                )
                    return
                if variant == "tap_outer":
                    for n in range(b_loc):
                        for j in range(2):
                            pss = [
                                psum.tile([128, RB, W], F32, tag="ps",
                                          name=f"ps_{n}_{j}_{r}")
                                for r in range(NRB)
                            ]
                            for ky in range(3):
                                for kx in range(3):
                                    for i in range(2):
                                        first = ky == 0 and kx == 0 and i == 0
                                        last = ky == 2 and kx == 2 and i == 1
                                        for r in range(NRB):
                                            nc.tensor.matmul(
                                                pss[r][:],
                                                w_sb[i][:, ky * 3 + kx, j, :],
                                                x_sb[n][i][
                                                    :,
                                                    r * RB + ky : r * RB + ky + RB,
                                                    kx : kx + W,
                                                ],
                                                start=first,
                                                stop=last,
                                            )
                            for r in range(NRB):
                                o = opool.tile([128, RB, W], F32, tag="o")
                                nc.scalar.activation(
                                    o[:], pss[r][:],
                                    mybir.ActivationFunctionType.Silu,
                                    bias=b_sb[j][:], scale=a_sb[j][:],
                                )
                                nc.sync.dma_start(
                                    out_d[n, j, :, r * RB : r * RB + RB, :],
                                    o[:],
                                )
                    return
                for n in range(b_loc):
                    for j in range(2):
                        for r in range(NRB):
                            ps = psum.tile([128, RB, W], F32, tag="ps")
                            idx = 0
                            for ky in range(3):
                                for kx in range(3):
                                    for i in range(2):
                                        nc.tensor.matmul(
                                            ps[:],
                                            w_sb[i][:, ky * 3 + kx, j, :],
                                            x_sb[n][i][
                                                :,
                                                r * RB + ky : r * RB + ky + RB,
                                                kx : kx + W,
                                            ],
                                            start=(idx == 0),
                                            stop=(idx == 17),
                                        )
                                        idx += 1
                            if variant == "no_act":
                                continue
                            o = opool.tile([128, RB, W], F32, tag="o")
                            nc.scalar.activation(
                                o[:], ps[:],
                                mybir.ActivationFunctionType.Silu,
                                bias=b_sb[j][:], scale=a_sb[j][:],
                            )
                            nc.sync.dma_start(
                                out_d[n, j, :, r * RB : r * RB + RB, :], o[:]
                            )

            if hw_loop:
                n_iter, n_body = repeats
                with tc.For_i(0, n_iter):
                    for _ in range(n_body):
                        body()
            else:
                for _rep in range(repeats):
                    body()
    if variant == "strip_ldw":
        # timing probe only: remove every InstLdweights (numerics garbage)
        for f in nc.m.functions:
            for bb in f.blocks:
                keep = []
                pending_waits = []
                for inst in bb.instructions:
                    if type(inst).__name__ == "InstLdweights":
                        si = inst.sync_info
                        if si and si.on_wait:
                            pending_waits.extend(si.on_wait)
                        continue
                    if pending_waits:
                        si = inst.sync_info
                        if si is None:
                            inst.sync_info = mybir.SyncInfo(
                                on_wait=pending_waits, on_update=[]
                            )
                        else:
                            si.on_wait = list(si.on_wait) + pending_waits
                        pending_waits = []
                    keep.append(inst)
                bb.instructions[:] = keep
    if do_split:
        split_sync_waits(nc)
    return nc


def preprocess(x, weight, gamma, beta, running_mean, running_var):
    """Host-side prep: ternarize, fold BN + ternary scale, pad/pack/cast."""
    x = np.asarray(x, dtype=np.float32)
    w = np.asarray(weight, dtype=np.float32)
    gamma = np.asarray(gamma, dtype=np.float32)
    beta = np.asarray(beta, dtype=np.float32)
    rm = np.asarray(running_mean, dtype=np.float32)
    rv = np.asarray(running_var, dtype=np.float32)

    s = np.float32(np.median(np.abs(w)))
    s_c = np.maximum(s, np.float32(1e-5))        # 1/scale of the reference
    scale = np.float32(1.0) / s_c
    t = np.clip(np.round(w * scale), -1.0, 1.0).astype(np.float32)

    inv = gamma / np.sqrt(rv + np.float32(1e-5))
    a = (s_c * inv).astype(np.float32)
    b = (beta - rm * inv).astype(np.float32)

    # [C2, C1, 3, 3] -> [i(c1 chunk), c1in, tap, j(c2 chunk), c2in]
    wp = (
        t.reshape(2, 128, 2, 128, 3, 3)
        .transpose(2, 3, 4, 5, 0, 1)
        .reshape(2, 128, 9, 2, 128)
        .astype(NP_X16)
    )
    ab = np.stack([a.reshape(2, 128), b.reshape(2, 128)], axis=-1).astype(
        np.float32
    )

    xp = np.zeros((B, 2, 128, HP, WP), dtype=NP_X16)
    xp[:, :, :, 1 : H + 1, 1 : W + 1] = x.reshape(B, 2, 128, H, W).astype(NP_X16)

    # winograd F(2,3) weights: G @ w over the kx dim; halves of ternary
    # sums -> exact in fp16.  gw[k, ky, c2, c1]
    gw = np.empty((4, 3) + t.shape[:2], dtype=np.float32)
    for ky in range(3):
        g = t[:, :, ky, :]
        gw[0, ky] = g[..., 0]
        gw[1, ky] = (g[..., 0] + g[..., 1] + g[..., 2]) * 0.5
        gw[2, ky] = (g[..., 0] - g[..., 1] + g[..., 2]) * 0.5
        gw[3, ky] = g[..., 2]
    # -> [i, c1in, k, ky, j, c2in]
    w2 = (
        gw.reshape(4, 3, 2, 128, 2, 128)
        .transpose(4, 5, 0, 1, 2, 3)
        .astype(NP_X16)
    )
    # fp8 DoubleRow weights for the k=0 stream: [c1in, ky, j, i, c2in]
    # GW0[ky] = t[:, :, ky, 0] (plain ternary -> exact in e4m3)
    w8 = (
        gw[0].reshape(3, 2, 128, 2, 128)      # [ky, j, c2in, i, c1in]
        .transpose(4, 0, 1, 3, 2)              # [c1in, ky, j, i, c2in]
        .astype(float8_e4m3fn)
    )
    w8b = (
        gw[3, 0].reshape(2, 128, 2, 128)       # [j, c2in, i, c1in]
        .transpose(3, 0, 2, 1)                  # [c1in, j, i, c2in]
        .astype(float8_e4m3fn)
    )
    return {"xp": xp, "wp": wp, "ab": ab,
            "w2": np.ascontiguousarray(w2),
            "w8": np.ascontiguousarray(w8),
            "w8b": np.ascontiguousarray(w8b)}


_NC_CACHE = {}

PROD_VARIANT = "wino_f8"


def get_nc(repeats=1, hw_loop=False, variant=None):
    v = PROD_VARIANT if variant is None else variant
    key = (repeats if not hw_loop else ("hw",) + tuple(repeats), v)
    if key not in _NC_CACHE:
        _NC_CACHE[key] = build_nc(B_LOC, repeats=repeats, hw_loop=hw_loop,
                                  variant=v)
    return _NC_CACHE[key]


def make_in_maps(prep):
    # dim-0 slices of a C-contiguous array are already contiguous
    xp = prep["xp"]
    rest = {k: v for k, v in prep.items() if k != "xp"}
    return [
        {"xp": xp[c * B_LOC : (c + 1) * B_LOC], **rest}
        for c in range(N_CORES)
    ]


def kernel(x, weight, gamma, beta, running_mean, running_var):
    prep = preprocess(x, weight, gamma, beta, running_mean, running_var)
    nc = get_nc()
    in_maps = make_in_maps(prep)
    # One retry: transient axon-mesh desync / wedged-core errors clear on a
    # fresh attempt (observed repeatedly in this environment).
    try:
        res = run_bass_kernel_spmd(nc, in_maps, list(range(N_CORES)))
    except Exception:
        import time as _time

        _time.sleep(3.0)
        res = run_bass_kernel_spmd(nc, in_maps, list(range(N_CORES)))
    return np.concatenate(
        [r["out"].reshape(B_LOC, C, H, W) for r in res.results], axis=0
    )



# revision 35
# speedup vs baseline: 1.3035x; 1.0330x over previous
"""nn_BitConv: ternary 3x3 conv (stride 1, pad 1) + BatchNorm(eval) + SiLU
on 8 Trainium2 NeuronCores, data-parallel over the batch dimension.

Strategy
--------
Host (numpy, negligible cost): ternarize the weight exactly like the
reference (scale = 1/median|w|, w_q = clamp(round(w*scale))/scale) and keep
only the integer part t in {-1,0,+1} (exact in fp16); fold the 1/scale
factor and the BatchNorm affine into a single per-output-channel
scale/shift (a, b). Pre-transpose the weight into the tensor-engine
stationary layout and zero-pad x to 58x58 / cast to fp16 (same PE rate as
bf16 but 10 mantissa bits: products with ternary weights are exact; only
the fp16 rounding of x contributes error, ~2e-4 relative on the output).

Device (per core, 4 images): the 3x3 conv is 9 shifted matmuls x 2
C1-chunks of K=128 accumulated in PSUM. For each image, C2-chunk (2x128)
and 8-row output block (7 per image), 18 matmuls of [K=128, M=128] x
[128, N=8*56=448] accumulate one PSUM tile; a single ScalarE activation
applies Silu(a*z + b) fused, then the tile is DMA'd out. 1008 back-to-back
matmuls keep the PE warm; ACT/DMA run concurrently. Measured ~180-220 us
per core (burst vs sustained-clock), at the 16-bit PE roofline for the
14.8 GFLOP/core conv.
"""
import numpy as np
from ml_dtypes import float8_e4m3fn
import concourse.bass as bass
from concourse import mybir
from concourse.bass_utils import run_bass_kernel_spmd
from concourse.tile import TileContext
from concourse.vector_clock import ScopedClock

X16 = mybir.dt.float16
F32 = mybir.dt.float32
NP_X16 = np.float16

N_CORES = 8
B, C, H, W = 32, 256, 56, 56
B_LOC = B // N_CORES
HP, WP = H + 2, W + 2
RB = 8            # output rows per PSUM tile (N = 8*56 = 448 <= 512)
NRB = H // RB
YB = 14           # winograd: output rows per PSUM tile (N = 14*28 = 392)
NYB = H // YB
TW = W // 2       # winograd F(2,3) output pairs per row


class _SplitDrainTC(TileContext):
    """This walrus build allows a single sync wait on the SP CTRL (Drain)
    instruction; split the Tile tail drain's waits across extra drains."""

    def _drain_and_barrier(self, tick_clock, wait_clock):
        drain_inst = self.nc.sync.drain()
        wait_clock.add_sem_waits(
            drain_inst.ins, ScopedClock({None: tick_clock.global_clock})
        )
        si = drain_inst.ins.sync_info
        waits = list(si.on_wait or []) if si is not None else []
        if len(waits) > 1:
            si.on_wait = waits[:1]
            for k in range(1, len(waits)):
                d2 = self.nc.sync.drain()
                si2 = d2.ins.sync_info
                if si2 is None:
                    d2.ins.sync_info = mybir.SyncInfo(
                        on_wait=[waits[k]], on_update=[]
                    )
                else:
                    si2.on_wait = [waits[k]]
        self.nc.all_engine_barrier()
        assert self.sems is not None
        popped = self.nc._tile_sem_poison_stack.pop()
        assert popped is self._sem_poison
        self.nc.clear_and_free_semaphores(list(self.sems.allocated().values()))
        self.nc.all_engine_barrier()


def split_sync_waits(nc, limit=1):
    """Hoist excess per-instruction sem waits onto same-engine nops (this
    walrus build allows only `limit` sync waits per instruction)."""
    builders = {
        mybir.EngineType.PE: nc.tensor,
        mybir.EngineType.Activation: nc.scalar,
        mybir.EngineType.DVE: nc.vector,
        mybir.EngineType.Pool: nc.gpsimd,
        mybir.EngineType.SP: nc.sync,
    }
    n_split = 0
    for f in nc.m.functions:
        for bb in f.blocks:
            insts = bb.instructions
            idx = 0
            while idx < len(insts):
                inst = insts[idx]
                si = inst.sync_info
                waits = list(si.on_wait) if (si is not None and si.on_wait) else []
                if len(waits) <= limit:
                    idx += 1
                    continue
                eng = inst.engine
                if eng not in builders:
                    raise RuntimeError(
                        f"split_sync_waits: no builder for engine {eng} "
                        f"on {inst.name} ({type(inst).__name__})"
                    )
                si.on_wait = waits[-limit:]
                carriers = []
                for w in waits[:-limit]:
                    nop = builders[eng].nop(nofuse=True)
                    ci = nop.ins
                    tail_bb = nc.cur_bb.bb
                    assert tail_bb.instructions[-1] is ci
                    tail_bb.instructions.pop()
                    ci.sync_info = mybir.SyncInfo(on_wait=[w], on_update=[])
                    carriers.append(ci)
                for k, ci in enumerate(carriers):
                    insts.insert(idx + k, ci)
                n_split += 1
                idx += len(carriers) + 1
    return n_split


def build_wino_nc(b_loc=B_LOC, repeats=1, do_split=True, hw_loop=False,
                  v_engines=("vector", "gpsimd"), skip_inverse=False,
                  fp8_k0=False, out16=False, fp8_extra=()):
    """1D Winograd F(2,3) along W: 12 matmul streams (4 k-terms x 3 ky) of
    K=256 replace the 18 direct-tap streams -> 2/3 of the PE row traffic.
    V transform (4 shifted add/subs of x) on DVE+Pool; inverse transform
    A^T (2 adds + 2 subs of the four M_k PSUM tiles) on DVE; BN+SiLU on ACT."""
    nc = bass.Bass()
    xp_d = nc.dram_tensor("xp", [b_loc, 2, 128, HP, WP], X16, kind="ExternalInput")
    w2_d = nc.dram_tensor("w2", [2, 128, 4, 3, 2, 128], X16, kind="ExternalInput")
    ab_d = nc.dram_tensor("ab", [2, 128, 2], F32, kind="ExternalInput")
    F8 = mybir.dt.float8e4
    out_d = nc.dram_tensor("out", [b_loc, 2, 128, H, W],
                           X16 if out16 else F32, kind="ExternalOutput")
    TWP = 32  # fp8 V plane: tw padded to 32 so the DoubleRow ktile stride
    #           (HP*TWP elems) is 16-byte aligned
    if fp8_k0:
        w8_d = nc.dram_tensor("w8", [128, 3, 2, 2, 128], F8,
                              kind="ExternalInput")
    if fp8_extra:
        w8b_d = nc.dram_tensor("w8b", [128, 2, 2, 128], F8,
                               kind="ExternalInput")

    with _SplitDrainTC(nc) as tc:
        with (
            tc.tile_pool(name="consts", bufs=1) as consts,
            tc.tile_pool(name="xpool", bufs=1) as xpool,
            tc.tile_pool(name="vpool", bufs=2) as vpool,
            tc.tile_pool(name="psum", bufs=8, space="PSUM") as psum,
            tc.tile_pool(name="tpool", bufs=4) as tpool,
            tc.tile_pool(name="ypool", bufs=3) as ypool,
            tc.tile_pool(name="opool", bufs=3) as opool,
        ):
            w_sb = []
            for i in range(2):
                w = consts.tile([128, 4, 3, 2, 128], X16, tag=f"w{i}")
                nc.sync.dma_start(w[:], w2_d[i])
                w_sb.append(w)
            if fp8_k0:
                w8_sb = consts.tile([128, 3, 2, 2, 128], F8, tag="w8")
                nc.sync.dma_start(w8_sb[:], w8_d[:])
            if fp8_extra:
                w8b_sb = consts.tile([128, 2, 2, 128], F8, tag="w8b")
                nc.sync.dma_start(w8b_sb[:], w8b_d[:])
            a_sb, b_sb = [], []
            for j in range(2):
                a = consts.tile([128, 1], F32, tag=f"a{j}")
                nc.sync.dma_start(a[:], ab_d[j, :, 0:1])
                a_sb.append(a)
                bt = consts.tile([128, 1], F32, tag=f"b{j}")
                nc.sync.dma_start(bt[:], ab_d[j, :, 1:2])
                b_sb.append(bt)
            x_sb = [[None] * 2 for _ in range(b_loc)]
            for n in range(b_loc):
                for i in range(2):
                    xt = xpool.tile([128, HP, WP], X16, tag=f"x{n}_{i}")
                    nc.sync.dma_start(xt[:], xp_d[n, i])
                    x_sb[n][i] = xt

            def make_v(n):
                """V[k, y, tw] combos; DVE does i=0 plane, Pool does i=1.
                With fp8_k0, the k=0 plane goes to a shared fp8 tile
                [128, 2(i), HP, TWP] consumed by DoubleRow matmuls."""
                vs = []
                v8 = None
                v8b = None
                if fp8_k0:
                    v8 = vpool.tile([128, 2, HP, TWP], F8, tag="v8",
                                    name=f"v8_{n}")
                if fp8_extra:
                    v8b = vpool.tile([128, 2, HP, TWP], F8, tag="v8b",
                                     name=f"v8b_{n}")
                for i in range(2):
                    eng = getattr(nc, v_engines[i])
                    v = vpool.tile([128, 4 - (1 if fp8_k0 else 0), HP, TW],
                                   X16, tag=f"v{i}", name=f"v_{n}_{i}")
                    xs = x_sb[n][i]

                    def sl(c0):
                        return xs[:, :, c0 : c0 + 2 * TW - 1 : 2]

                    if fp8_k0:
                        eng.tensor_sub(v8[:, i, :, 0:TW], sl(0), sl(2))
                        eng.tensor_add(v[:, 0], sl(1), sl(2))
                        eng.tensor_sub(v[:, 1], sl(2), sl(1))
                        eng.tensor_sub(v[:, 2], sl(1), sl(3))
                        if fp8_extra:
                            eng.tensor_sub(v8b[:, i, :, 0:TW], sl(1), sl(3))
                    else:
                        eng.tensor_sub(v[:, 0], sl(0), sl(2))
                        eng.tensor_add(v[:, 1], sl(1), sl(2))
                        eng.tensor_sub(v[:, 2], sl(2), sl(1))
                        eng.tensor_sub(v[:, 3], sl(1), sl(3))
                    vs.append(v)
                return (vs, v8, v8b)

            def mm_block(n, v_all):
                v_n, v8_n, v8b_n = v_all
                for j in range(2):
                    for blk in range(NYB):
                        pss = []
                        if fp8_k0:
                            ps = psum.tile([128, YB, TW], F32, tag="ps",
                                           name=f"ps8_{n}_{j}_{blk}")
                            for ky in range(3):
                                r0 = blk * YB + ky
                                nc.tensor.matmul(
                                    ps[:],
                                    w8_sb[:, ky, j, :, :],
                                    v8_n[:, :, r0 : r0 + YB, 0:TW],
                                    perf_mode=mybir.MatmulPerfMode.DoubleRow,
                                    start=(ky == 0),
                                    stop=(ky == 2),
                                )
                            pss.append(ps)
                        k_lo = 1 if fp8_k0 else 0
                        for k in range(k_lo, 4):
                            ps = psum.tile([128, YB, TW], F32, tag="ps",
                                           name=f"ps_{n}_{j}_{blk}_{k}")
                            fp8_kys = (
                                (0,) if (fp8_extra and k == 3) else ()
                            )
                            for ky in fp8_kys:
                                r0 = blk * YB + ky
                                nc.tensor.matmul(
                                    ps[:],
                                    w8b_sb[:, j, :, :],
                                    v8b_n[:, :, r0 : r0 + YB, 0:TW],
                                    perf_mode=mybir.MatmulPerfMode.DoubleRow,
                                    start=(ky == 0),
                                    stop=False,
                                )
                            for ky in range(3):
                                if ky in fp8_kys:
                                    continue
                                for i in range(2):
                                    nc.tensor.matmul(
                                        ps[:],
                                        w_sb[i][:, k, ky, j, :],
                                        v_n[i][
                                            :, k - k_lo,
                                            blk * YB + ky : blk * YB + ky + YB,
                                            :,
                                        ],
                                        start=(ky == 0 and i == 0
                                               and not fp8_kys),
                                        stop=(ky == 2 and i == 1),
                                    )
                            pss.append(ps)
                        if skip_inverse:
                            continue
                        # ACT drains all four M_k tiles to SBUF as fp16
                        # (frees the PSUM banks without waiting on DVE);
                        # DVE computes the A^T combos from contiguous fp16
                        # tiles in 2x mode; ACT's two SiLU calls interleave
                        # even/odd output columns (ACT is 1x regardless).
                        ms = []
                        for k in range(4):
                            m = tpool.tile([128, YB, TW], X16, tag=f"m{k}",
                                           name=f"m_{n}_{j}_{blk}_{k}")
                            nc.scalar.copy(m[:], pss[k][:])
                            ms.append(m)
                        te = tpool.tile([128, YB, TW], X16, tag="te")
                        to = tpool.tile([128, YB, TW], X16, tag="to")
                        ye = ypool.tile([128, YB, TW], X16, tag="ye")
                        yo = ypool.tile([128, YB, TW], X16, tag="yo")
                        nc.vector.tensor_add(te[:], ms[0][:], ms[1][:])
                        nc.vector.tensor_sub(to[:], ms[1][:], ms[2][:])
                        nc.vector.tensor_add(ye[:], te[:], ms[2][:])
                        nc.vector.tensor_sub(yo[:], to[:], ms[3][:])
                        oo = opool.tile([128, YB, W],
                                        X16 if out16 else F32, tag="o")
                        nc.scalar.activation(
                            oo[:, :, 0::2], ye[:],
                            mybir.ActivationFunctionType.Silu,
                            bias=b_sb[j][:], scale=a_sb[j][:],
                        )
                        nc.scalar.activation(
                            oo[:, :, 1::2], yo[:],
                            mybir.ActivationFunctionType.Silu,
                            bias=b_sb[j][:], scale=a_sb[j][:],
                        )
                        nc.sync.dma_start(
                            out_d[n, j, :, blk * YB : blk * YB + YB, :], oo[:]
                        )

            def body():
                v_tiles = [None] * b_loc
                v_tiles[0] = make_v(0)
                for n in range(b_loc):
                    if n + 1 < b_loc:
                        v_tiles[n + 1] = make_v(n + 1)
                    mm_block(n, v_tiles[n])

            if hw_loop:
                n_iter, n_body = repeats
                with tc.For_i(0, n_iter):
                    for _ in range(n_body):
                        body()
            else:
                for _rep in range(repeats):
                    body()
    if do_split:
        split_sync_waits(nc)
    return nc




NQ = W // 4       # F(4,3): output quads per row
QP = 16           # padded quad dim in the phase-split layout


def build_wino43_nc(b_loc=B_LOC, repeats=1, do_split=True, hw_loop=False,
                    skip_inverse=False):
    """1D Winograd F(4,3) along W: 18 matmul streams (6 t-terms x 3 ky) of
    N=196 replace F(2,3)'s 24 -> 0.75x of its PE row traffic (0.5x of
    direct).  Input arrives host-split into 6 phase planes so the B^T
    combos are contiguous fp16 ops (DVE 2x mode); inverse A^T runs on
    fp16 in 2x mode; ACT interleaves the four output phases via strided
    SiLU writes."""
    nc = bass.Bass()
    xq_d = nc.dram_tensor("xq", [b_loc, 2, 128, 6, HP, QP], X16,
                          kind="ExternalInput")
    w6_d = nc.dram_tensor("w6", [2, 128, 6, 3, 2, 128], X16,
                          kind="ExternalInput")
    ab_d = nc.dram_tensor("ab", [2, 128, 2], F32, kind="ExternalInput")
    out_d = nc.dram_tensor("out", [b_loc, 2, 128, H, W], F32,
                           kind="ExternalOutput")
    AOp = mybir.AluOpType

    with _SplitDrainTC(nc) as tc:
        with (
            tc.tile_pool(name="consts", bufs=1) as consts,
            tc.tile_pool(name="xpool", bufs=1) as xpool,
            tc.tile_pool(name="vtmp", bufs=1) as vtmp,
            tc.tile_pool(name="vpool", bufs=2) as vpool,
            tc.tile_pool(name="psum", bufs=8, space="PSUM") as psum,
            tc.tile_pool(name="tpool", bufs=3) as tpool,
            tc.tile_pool(name="ypool", bufs=2) as ypool,
            tc.tile_pool(name="opool", bufs=2) as opool,
        ):
            w_sb = []
            for i in range(2):
                w = consts.tile([128, 6, 3, 2, 128], X16, tag=f"w{i}")
                nc.sync.dma_start(w[:], w6_d[i])
                w_sb.append(w)
            a_sb, b_sb = [], []
            for j in range(2):
                a = consts.tile([128, 1], F32, tag=f"a{j}")
                nc.sync.dma_start(a[:], ab_d[j, :, 0:1])
                a_sb.append(a)
                bt = consts.tile([128, 1], F32, tag=f"b{j}")
                nc.sync.dma_start(bt[:], ab_d[j, :, 1:2])
                b_sb.append(bt)
            x_sb = [[None] * 2 for _ in range(b_loc)]
            for n in range(b_loc):
                for i in range(2):
                    xt = xpool.tile([128, 6, HP, QP], X16, tag=f"x{n}_{i}")
                    nc.sync.dma_start(xt[:], xq_d[n, i])
                    x_sb[n][i] = xt

            def make_v(n):
                vs = []
                for i in range(2):
                    X = x_sb[n][i]
                    ph = [X[:, d] for d in range(6)]
                    v = vpool.tile([128, 6, HP, QP], X16, tag=f"v{i}",
                                   name=f"v_{n}_{i}")

                    def tmp(name):
                        return vtmp.tile([128, HP, QP], X16, tag=name,
                                         name=f"{name}_{n}_{i}")

                    stt = nc.vector.scalar_tensor_tensor
                    u = tmp("u")
                    stt(u[:], ph[2], -1.25, ph[0], AOp.mult, AOp.add)
                    stt(v[:, 0], u[:], 4.0, ph[4], AOp.mult, AOp.add)
                    aa = tmp("aa"); bb2 = tmp("bb2")
                    nc.vector.tensor_add(aa[:], ph[1], ph[2])
                    nc.vector.tensor_add(bb2[:], ph[3], ph[4])
                    stt(v[:, 1], aa[:], -4.0, bb2[:], AOp.mult, AOp.add)
                    cc = tmp("cc"); dd = tmp("dd")
                    nc.vector.tensor_sub(cc[:], ph[1], ph[2])
                    nc.vector.tensor_sub(dd[:], ph[4], ph[3])
                    stt(v[:, 2], cc[:], 4.0, dd[:], AOp.mult, AOp.add)
                    ee = tmp("ee"); ff = tmp("ff")
                    nc.vector.tensor_sub(ee[:], ph[1], ph[3])
                    nc.vector.tensor_sub(ff[:], ph[4], ph[2])
                    stt(v[:, 3], ee[:], -2.0, ff[:], AOp.mult, AOp.add)
                    stt(v[:, 4], ee[:], 2.0, ff[:], AOp.mult, AOp.add)
                    u2 = tmp("u2")
                    stt(u2[:], ph[3], -1.25, ph[1], AOp.mult, AOp.add)
                    stt(v[:, 5], u2[:], 4.0, ph[5], AOp.mult, AOp.add)
                    vs.append(v)
                return vs

            YB4 = 28
            def mm_block(n, v_n):
                for j in range(2):
                    for blk in range(2):
                        pss = []
                        for t in range(6):
                            ps = psum.tile([128, YB4, NQ], F32, tag="ps",
                                           name=f"ps_{n}_{j}_{blk}_{t}")
                            for ky in range(3):
                                for i in range(2):
                                    nc.tensor.matmul(
                                        ps[:],
                                        w_sb[i][:, t, ky, j, :],
                                        v_n[i][
                                            :, t,
                                            blk * YB4 + ky : blk * YB4 + ky
                                            + YB4,
                                            0:NQ,
                                        ],
                                        start=(ky == 0 and i == 0),
                                        stop=(ky == 2 and i == 1),
                                    )
                            pss.append(ps)
                        if skip_inverse:
                            continue
                        ms = []
                        for t in range(6):
                            m = tpool.tile([128, YB4, NQ], X16, tag=f"m{t}",
                                           name=f"m_{n}_{j}_{blk}_{t}")
                            nc.scalar.copy(m[:], pss[t][:])
                            ms.append(m)
                        stt = nc.vector.scalar_tensor_tensor

                        def yt(name):
                            return ypool.tile([128, YB4, NQ], X16, tag=name,
                                              name=f"{name}_{n}_{j}_{blk}")

                        s4 = yt("s4"); s5 = yt("s5"); t0 = yt("t0")
                        y0 = yt("y0"); d1 = yt("d1"); d2 = yt("d2")
                        y1 = yt("y1"); t2 = yt("t2"); y3 = yt("y3")
                        y2 = yt("y2")
                        nc.vector.tensor_add(s4[:], ms[1][:], ms[2][:])
                        nc.vector.tensor_add(s5[:], ms[3][:], ms[4][:])
                        nc.vector.tensor_add(t0[:], ms[0][:], s4[:])
                        nc.vector.tensor_add(y0[:], t0[:], s5[:])
                        nc.vector.tensor_sub(d1[:], ms[1][:], ms[2][:])
                        nc.vector.tensor_sub(d2[:], ms[3][:], ms[4][:])
                        stt(y1[:], d2[:], 2.0, d1[:], AOp.mult, AOp.add)
                        stt(t2[:], d2[:], 8.0, d1[:], AOp.mult, AOp.add)
                        nc.vector.tensor_add(y3[:], t2[:], ms[5][:])
                        stt(y2[:], s5[:], 4.0, s4[:], AOp.mult, AOp.add)
                        oo = opool.tile([128, YB4, W], F32, tag="o")
                        for c, yy in enumerate((y0, y1, y2, y3)):
                            nc.scalar.activation(
                                oo[:, :, c::4], yy[:],
                                mybir.ActivationFunctionType.Silu,
                                bias=b_sb[j][:], scale=a_sb[j][:],
                            )
                        nc.sync.dma_start(
                            out_d[n, j, :, blk * YB4 : blk * YB4 + YB4, :], oo[:]
                        )

            def body():
                v_tiles = [None] * b_loc
                v_tiles[0] = make_v(0)
                for n in range(b_loc):
                    if n + 1 < b_loc:
                        v_tiles[n + 1] = make_v(n + 1)
                    mm_block(n, v_tiles[n])

            if hw_loop:
                n_iter, n_body = repeats
                with tc.For_i(0, n_iter):
                    for _ in range(n_body):
                        body()
            else:
                for _rep in range(repeats):
                    body()
    if do_split:
        split_sync_waits(nc)
    return nc

def build_nc(b_loc=B_LOC, repeats=1, do_split=True, hw_loop=False,
             variant="base"):
    if variant == "wino43":
        return build_wino43_nc(b_loc, repeats, do_split, hw_loop)
    if variant == "wino43_noinv":
        return build_wino43_nc(b_loc, repeats, do_split, hw_loop,
                               skip_inverse=True)
    if variant == "wino":
        return build_wino_nc(b_loc, repeats, do_split, hw_loop)
    if variant == "wino_dve":
        return build_wino_nc(b_loc, repeats, do_split, hw_loop,
                             v_engines=("vector", "vector"))
    if variant == "wino_gps":
        return build_wino_nc(b_loc, repeats, do_split, hw_loop,
                             v_engines=("gpsimd", "gpsimd"))
    if variant == "wino_noinv":
        return build_wino_nc(b_loc, repeats, do_split, hw_loop,
                             v_engines=("vector", "vector"),
                             skip_inverse=True)
    if variant == "wino_f8":
        return build_wino_nc(b_loc, repeats, do_split, hw_loop,
                             v_engines=("vector", "vector"), fp8_k0=True)
    if variant == "wino_f8_o16":
        return build_wino_nc(b_loc, repeats, do_split, hw_loop,
                             v_engines=("vector", "vector"), fp8_k0=True,
                             out16=True)
    if variant == "wino_f8_noinv":
        return build_wino_nc(b_loc, repeats, do_split, hw_loop,
                             v_engines=("vector", "vector"), fp8_k0=True,
                             skip_inverse=True)
    if variant == "wino_f8x":
        return build_wino_nc(b_loc, repeats, do_split, hw_loop,
                             v_engines=("vector", "vector"), fp8_k0=True,
                             fp8_extra=((3, 0),))
    nc = bass.Bass()
    xp_d = nc.dram_tensor("xp", [b_loc, 2, 128, HP, WP], X16, kind="ExternalInput")
    wp_d = nc.dram_tensor("wp", [2, 128, 9, 2, 128], X16, kind="ExternalInput")
    ab_d = nc.dram_tensor("ab", [2, 128, 2], F32, kind="ExternalInput")
    out_d = nc.dram_tensor("out", [b_loc, 2, 128, H, W], F32, kind="ExternalOutput")

    with _SplitDrainTC(nc) as tc:
        with (
            tc.tile_pool(name="consts", bufs=1) as consts,
            tc.tile_pool(name="xpool", bufs=1) as xpool,
            tc.tile_pool(name="psum", bufs=8, space="PSUM") as psum,
            tc.tile_pool(name="opool", bufs=4) as opool,
        ):
            w_sb = []
            for i in range(2):
                w = consts.tile([128, 9, 2, 128], X16, tag=f"w{i}")
                nc.sync.dma_start(w[:], wp_d[i])
                w_sb.append(w)
            a_sb, b_sb = [], []
            for j in range(2):
                a = consts.tile([128, 1], F32, tag=f"a{j}")
                nc.sync.dma_start(a[:], ab_d[j, :, 0:1])
                a_sb.append(a)
                bt = consts.tile([128, 1], F32, tag=f"b{j}")
                nc.sync.dma_start(bt[:], ab_d[j, :, 1:2])
                b_sb.append(bt)
            x_sb = [[None] * 2 for _ in range(b_loc)]
            for n in range(b_loc):
                for i in range(2):
                    xt = xpool.tile([128, HP, WP], X16, tag=f"x{n}_{i}")
                    nc.sync.dma_start(xt[:], xp_d[n, i])
                    x_sb[n][i] = xt

            def body():
                if variant in ("small_n", "tiny_n"):
                    rb = 4 if variant == "small_n" else 2
                    nrb = H // rb
                    for n in range(b_loc):
                        for j in range(2):
                            for r in range(nrb):
                                ps = psum.tile([128, rb, W], F32, tag="ps")
                                idx = 0
                                for ky in range(3):
                                    for kx in range(3):
                                        for i in range(2):
                                            nc.tensor.matmul(
                                                ps[:],
                                                w_sb[i][:, ky * 3 + kx, j, :],
                                                x_sb[n][i][
                                                    :,
                                                    r * rb + ky : r * rb + ky + rb,
                                                    kx : kx + W,
                                                ],
                                                start=(idx == 0),
                                                stop=(idx == 17),
                                            )
                                            idx += 1
                                o = opool.tile([128, rb, W], F32, tag="o")
                                nc.scalar.activation(
                                    o[:], ps[:],
                                    mybir.ActivationFunctionType.Silu,
                                    bias=b_sb[j][:], scale=a_sb[j][:],
                                )
                                nc.sync.dma_start(
                                    out_d[n, j, :, r * rb : r * rb + rb, :],
                                    o[:],
                                )
                    return
                if variant == "ldw":
                    # explicit weight load once per (tap, i); matmuls flagged
                    # ldweights=False reuse the loaded stationary operand
                    for n in range(b_loc):
                        for j in range(2):
                            pss = [
                                psum.tile([128, RB, W], F32, tag="ps",
                                          name=f"psl_{n}_{j}_{r}")
                                for r in range(NRB)
                            ]
                            for ky in range(3):
                                for kx in range(3):
                                    for i in range(2):
                                        first = ky == 0 and kx == 0 and i == 0
                                        last = ky == 2 and kx == 2 and i == 1
                                        wap = w_sb[i][:, ky * 3 + kx, j, :]
                                        nc.tensor.ldweights(wap)
                                        for r in range(NRB):
                                            h = nc.tensor.matmul(
                                                pss[r][:],
                                                wap,
                                                x_sb[n][i][
                                                    :,
                                                    r * RB + ky : r * RB + ky + RB,
                                                    kx : kx + W,
                                                ],
                                                start=first,
                                                stop=last,
                                            )
                                            h.ins.ldweights = False
                            for r in range(NRB):
                                o = opool.tile([128, RB, W], F32, tag="o")
                                nc.scalar.activation(
                                    o[:], pss[r][:],
                                    mybir.ActivationFunctionType.Silu,
                                    bias=b_sb[j][:], scale=a_sb[j][:],
                                )
                                nc.sync.dma_start(
                                    out_d[n, j, :, r * RB : r * RB + RB, :],
                                    o[:],
                                )
                    return
                if variant == "tap_outer":
                    for n in range(b_loc):
                        for j in range(2):
                            pss = [
                                psum.tile([128, RB, W], F32, tag="ps",
                                          name=f"ps_{n}_{j}_{r}")
                                for r in range(NRB)
                            ]
                            for ky in range(3):
                                for kx in range(3):
                                    for i in range(2):
                                        first = ky == 0 and kx == 0 and i == 0
                                        last = ky == 2 and kx == 2 and i == 1
                                        for r in range(NRB):
                                            nc.tensor.matmul(
                                                pss[r][:],
                                                w_sb[i][:, ky * 3 + kx, j, :],
                                                x_sb[n][i][
                                                    :,
                                                    r * RB + ky : r * RB + ky + RB,
                                                    kx : kx + W,
                                                ],
                                                start=first,
                                                stop=last,
                                            )
                            for r in range(NRB):
                                o = opool.tile([128, RB, W], F32, tag="o")
                                nc.scalar.activation(
                                    o[:], pss[r][:],
                                    mybir.ActivationFunctionType.Silu,
                                    bias=b_sb[j][:], scale=a_sb[j][:],
                                )
                                nc.sync.dma_start(
                                    out_d[n, j, :, r * RB : r * RB + RB, :],
                                    o[:],
                                )
                    return
                for n in range(b_loc):
                    for j in range(2):
                        for r in range(NRB):
                            ps = psum.tile([128, RB, W], F32, tag="ps")
                            idx = 0
                            for ky in range(3):
                                for kx in range(3):
                                    for i in range(2):
                                        nc.tensor.matmul(
                                            ps[:],
                                            w_sb[i][:, ky * 3 + kx, j, :],
                                            x_sb[n][i][
                                                :,
                                                r * RB + ky : r * RB + ky + RB,
                                                kx : kx + W,
                                            ],
                                            start=(idx == 0),
                                            stop=(idx == 17),
                                        )
                                        idx += 1
                            if variant == "no_act":
                                continue
                            o = opool.tile([128, RB, W], F32, tag="o")
                            nc.scalar.activation(
                                o[:], ps[:],
                                mybir.ActivationFunctionType.Silu,
                                bias=b_sb[j][:], scale=a_sb[j][:],
                            )
                            nc.sync.dma_start(
                                out_d[n, j, :, r * RB : r * RB + RB, :], o[:]
                            )

            if hw_loop:
                n_iter, n_body = repeats
                with tc.For_i(0, n_iter):
                    for _ in range(n_body):
                        body()
            else:
                for _rep in range(repeats):
                    body()
    if variant == "strip_ldw":
        # timing probe only: remove every InstLdweights (numerics garbage)
        for f in nc.m.functions:
            for bb in f.blocks:
                keep = []
                pending_waits = []
                for inst in bb.instructions:
                    if type(inst).__name__ == "InstLdweights":
                        si = inst.sync_info
                        if si and si.on_wait:
                            pending_waits.extend(si.on_wait)
                        continue
                    if pending_waits:
                        si = inst.sync_info
                        if si is None:
                            inst.sync_info = mybir.SyncInfo(
                                on_wait=pending_waits, on_update=[]
                            )
                        else:
                            si.on_wait = list(si.on_wait) + pending_waits
                        pending_waits = []
                    keep.append(inst)
                bb.instructions[:] = keep
    if do_split:
        split_sync_waits(nc)
    return nc


def preprocess(x, weight, gamma, beta, running_mean, running_var):
    """Host-side prep: ternarize, fold BN + ternary scale, pad/pack/cast."""
    x = np.asarray(x, dtype=np.float32)
    w = np.asarray(weight, dtype=np.float32)
    gamma = np.asarray(gamma, dtype=np.float32)
    beta = np.asarray(beta, dtype=np.float32)
    rm = np.asarray(running_mean, dtype=np.float32)
    rv = np.asarray(running_var, dtype=np.float32)

    s = np.float32(np.median(np.abs(w)))
    s_c = np.maximum(s, np.float32(1e-5))        # 1/scale of the reference
    scale = np.float32(1.0) / s_c
    t = np.clip(np.round(w * scale), -1.0, 1.0).astype(np.float32)

    inv = gamma / np.sqrt(rv + np.float32(1e-5))
    a = (s_c * inv).astype(np.float32)
    b = (beta - rm * inv).astype(np.float32)

    # [C2, C1, 3, 3] -> [i(c1 chunk), c1in, tap, j(c2 chunk), c2in]
    wp = (
        t.reshape(2, 128, 2, 128, 3, 3)
        .transpose(2, 3, 4, 5, 0, 1)
        .reshape(2, 128, 9, 2, 128)
        .astype(NP_X16)
    )
    ab = np.stack([a.reshape(2, 128), b.reshape(2, 128)], axis=-1).astype(
        np.float32
    )

    xp = np.zeros((B, 2, 128, HP, WP), dtype=NP_X16)
    xp[:, :, :, 1 : H + 1, 1 : W + 1] = x.reshape(B, 2, 128, H, W).astype(NP_X16)

    # winograd F(2,3) weights: G @ w over the kx dim; halves of ternary
    # sums -> exact in fp16.  gw[k, ky, c2, c1]
    gw = np.empty((4, 3) + t.shape[:2], dtype=np.float32)
    for ky in range(3):
        g = t[:, :, ky, :]
        gw[0, ky] = g[..., 0]
        gw[1, ky] = (g[..., 0] + g[..., 1] + g[..., 2]) * 0.5
        gw[2, ky] = (g[..., 0] - g[..., 1] + g[..., 2]) * 0.5
        gw[3, ky] = g[..., 2]
    # -> [i, c1in, k, ky, j, c2in]
    w2 = (
        gw.reshape(4, 3, 2, 128, 2, 128)
        .transpose(4, 5, 0, 1, 2, 3)
        .astype(NP_X16)
    )
    # fp8 DoubleRow weights for the k=0 stream: [c1in, ky, j, i, c2in]
    # GW0[ky] = t[:, :, ky, 0] (plain ternary -> exact in e4m3)
    w8 = (
        gw[0].reshape(3, 2, 128, 2, 128)      # [ky, j, c2in, i, c1in]
        .transpose(4, 0, 1, 3, 2)              # [c1in, ky, j, i, c2in]
        .astype(float8_e4m3fn)
    )
    w8b = (
        gw[3, 0].reshape(2, 128, 2, 128)       # [j, c2in, i, c1in]
        .transpose(3, 0, 2, 1)                  # [c1in, j, i, c2in]
        .astype(float8_e4m3fn)
    )

    # F(4,3): 6 phase planes (contiguous quad dim) + transformed weights
    NQl, QPl = W // 4, 16
    xpad = np.zeros((B, 2, 128, HP, WP), dtype=np.float32)
    xpad[:, :, :, 1 : H + 1, 1 : W + 1] = x.reshape(B, 2, 128, H, W)
    xpad16 = xpad.astype(NP_X16)
    xq = np.zeros((B, 2, 128, 6, HP, QPl), dtype=NP_X16)
    for d in range(6):
        xq[:, :, :, d, :, 0:NQl] = xpad16[:, :, :, :, d : d + 4 * (NQl - 1) + 1 : 4]
    g0, g1, g2 = t[..., 0], t[..., 1], t[..., 2]  # [C2, C1, ky] each
    gw6 = np.stack([
        g0 / 4.0,
        (-g0 - g1 - g2) / 6.0,
        (-g0 + g1 - g2) / 6.0,
        (g0 + 2 * g1 + 4 * g2) / 24.0,
        (g0 - 2 * g1 + 4 * g2) / 24.0,
        g2,
    ], axis=0).transpose(0, 3, 1, 2)               # [6, ky, C2, C1]
    w6 = (
        gw6.reshape(6, 3, 2, 128, 2, 128)
        .transpose(4, 5, 0, 1, 2, 3)                # [i, c1in, 6, ky, j, c2in]
        .astype(NP_X16)
    )
    return {"xp": xp, "wp": wp, "ab": ab,
            "w2": np.ascontiguousarray(w2),
            "w8": np.ascontiguousarray(w8),
            "w8b": np.ascontiguousarray(w8b),
            "xq": np.ascontiguousarray(xq),
            "w6": np.ascontiguousarray(w6)}


_NC_CACHE = {}

PROD_VARIANT = "wino_f8x"


def get_nc(repeats=1, hw_loop=False, variant=None):
    v = PROD_VARIANT if variant is None else variant
    key = (repeats if not hw_loop else ("hw",) + tuple(repeats), v)
    if key not in _NC_CACHE:
        _NC_CACHE[key] = build_nc(B_LOC, repeats=repeats, hw_loop=hw_loop,
                                  variant=v)
    return _NC_CACHE[key]


_SHARDED = ("xp", "xq")  # batch-sharded inputs; the rest are replicated


def make_in_maps(prep):
    # dim-0 slices of a C-contiguous array are already contiguous
    rest = {k: v for k, v in prep.items() if k not in _SHARDED}
    return [
        {
            **{k: prep[k][c * B_LOC : (c + 1) * B_LOC] for k in _SHARDED},
            **rest,
        }
        for c in range(N_CORES)
    ]


def kernel(x, weight, gamma, beta, running_mean, running_var):
    prep = preprocess(x, weight, gamma, beta, running_mean, running_var)
    nc = get_nc()
    in_maps = make_in_maps(prep)
    # One retry: transient axon-mesh desync / wedged-core errors clear on a
    # fresh attempt (observed repeatedly in this environment).
    try:
        res = run_bass_kernel_spmd(nc, in_maps, list(range(N_CORES)))
    except Exception:
        import time as _time

        _time.sleep(3.0)
        res = run_bass_kernel_spmd(nc, in_maps, list(range(N_CORES)))
    return np.concatenate(
        [r["out"].reshape(B_LOC, C, H, W) for r in res.results], axis=0
    )

